# revision 15
# baseline (speedup 1.0000x reference)
"""Trainium2 Bass kernel for nn_BDH_4406636445711 (dense transformer).

Sharding: 8 cores = data-parallel over B(2) x tensor-parallel over H(4).
Core c handles (b = c//4, h = c%4): its head's Dx/Dy slices, E rows, and a
V/4 shard of the readout. Per layer the y@E partial is AllReduced (bf16)
within each b-group of 4 cores. The host stitches the 8 per-core [VS, T]
logit shards (bf16 on device, cast to fp32 host-side) into [B, T, V].

Key algebraic optimization vs the naive graph: scores = q @ q^T is only
ever used for a = scores @ v, so we compute a = q @ (q^T v) instead --
G = q^T v is [K, D]; ~5x fewer PE cycles than materializing [T, T] scores.

Layouts: v lives token-major ("td": [128 tok part, 16 chunk, 256 d]) so
every LayerNorm is a free-dim reduction. x/q live kT; q is additionally
transposed to tk for the G matmul. All matmul operands are bf16 (full PE
rate); accumulation and LN stats stay fp32. ACT only ever needs the
{relu, copy, sqrt, square, identity} table -- no table reloads.

Scheduling notes:
- Both a-halves' matmuls are issued back-to-back so the first half's
  batched LN statistics overlap the second half's matmuls; the a-sums
  ride for free on the PSUM->SBUF copies via the ACT accumulator.
- v_td holds ln(w) WITHOUT pos; the +pos happens (a) fused into the
  vp_bf cast (one DVE add, bf16 out) and (b) at the top of the next
  E chain where it is latency-hidden. This shortens the E tail that
  gates the next phase-B matmuls.
- The layer is software-pipelined around the two AllReduces: E(th0)'s
  stat chain overlaps D(th1)'s matmuls; the next layer's B/G halves
  cover the second collective. The readout runs as two column-half
  passes so the first pass overlaps the last layer's E(th1) chain.
"""

import os
import sys

sys.path.insert(0, "/opt/trn_rl_repo")

import numpy as np

import concourse.bass as bass
import concourse.tile as tile
from concourse import bacc, mybir
from concourse.bass_utils import run_bass_kernel_spmd
from concourse.masks import make_identity
from concourse import library_config

F32 = mybir.dt.float32
BF16 = mybir.dt.bfloat16
I32 = mybir.dt.int32
AF = mybir.ActivationFunctionType
OP = mybir.AluOpType
AX = mybir.AxisListType

B, T, H, D, K, V, L = 2, 2048, 4, 256, 1024, 32000, 6
VS = V // 4          # vocab shard per core within a b-group
EPS = 1e-5
NT = T // 128        # 16 token chunks
NKC = K // 128       # 8 k chunks
ND = D // 128        # 2 d chunks
TH = T // 2          # 1024

N_LAYERS = int(os.environ.get("KRN_LAYERS", str(L)))
DO_READOUT = os.environ.get("KRN_READOUT", "1") == "1"


def build(nc):
    # ---- DRAM parameters (per core) ----
    tok_d = nc.dram_tensor("tok", [T], I32, kind="ExternalInput")
    emb_d = nc.dram_tensor("emb", [V, D], F32, kind="ExternalInput")
    pos_d = nc.dram_tensor("posb", [T, D], BF16, kind="ExternalInput")
    dx_d = nc.dram_tensor("dxb", [D, K], BF16, kind="ExternalInput")
    dy_d = nc.dram_tensor("dyb", [D, K], BF16, kind="ExternalInput")
    e_d = nc.dram_tensor("eb", [K, D], BF16, kind="ExternalInput")
    ro_d = nc.dram_tensor("rob", [D, VS], BF16, kind="ExternalInput")
    cos_d = nc.dram_tensor("cosb", [8, 128, TH], BF16, kind="ExternalInput")
    sin_d = nc.dram_tensor("sinb", [8, 128, TH], BF16, kind="ExternalInput")
    out_d = nc.dram_tensor("logitsT", [VS, T], BF16, kind="ExternalOutput")

    groups = [[0, 1, 2, 3], [4, 5, 6, 7]]

    with tile.TileContext(nc) as tc:
        with (
            nc.allow_low_precision(reason="bf16 matmul path is intentional"),
            tc.tile_pool(name="persist", bufs=1) as pp,
            tc.tile_pool(name="w4", bufs=4) as w4p,     # [128,1024] bf16 rope
            tc.tile_pool(name="stats", bufs=24) as stp, # [128,8] f32
            tc.tile_pool(name="rop", bufs=3) as rop,    # readout weights
            tc.tile_pool(name="lop", bufs=3) as lop,    # logit staging
            tc.tile_pool(name="pb", bufs=2, space="PSUM") as pbp,   # [128,1024] f32
            tc.tile_pool(name="pa", bufs=2, space="PSUM") as pap,   # [128,256] f32
            tc.tile_pool(name="pt", bufs=2, space="PSUM") as ptp,   # [128,1024] bf16
            tc.tile_pool(name="dram", bufs=1, space="DRAM") as dpool,
        ):
            _ctr = [0]

            def _nm(p):
                _ctr[0] += 1
                return f"{p}{_ctr[0]}"

            # ---- constants ----
            ident_f = pp.tile([128, 128], F32)
            make_identity(nc, ident_f[:])
            ident_bf = pp.tile([128, 128], BF16)
            nc.vector.tensor_copy(ident_bf[:], ident_f[:])
            eps_p = pp.tile([128, 1], F32)
            nc.vector.memset(eps_p[:], EPS)
            nc.gpsimd.load_library(library_config.attn)

            # ---- persistent tensors ----
            v_td = pp.tile([128, NT, D], F32)          # ln(w), token-major (no pos)
            vp_bf = pp.tile([128, NT, D], BF16)        # v + pos, bf16
            vpT = pp.tile([128, ND, T], BF16)          # (v+pos) transposed
            qT = pp.tile([128, NKC, T], BF16)          # q k-major; reused as yt
            qtk = pp.tile([128, NT, K], BF16)          # q token-major
            x_bf = pp.tile([128, NKC, T], BF16)        # relu(v@Dx), k-major
            g0_sb = pp.tile([128, NKC, D], BF16)       # G partial (th0 tokens)
            g_bf = pp.tile([128, NKC, D], BF16)        # G = q^T (v+pos), full
            lnA_dT = pp.tile([128, ND, T], BF16)       # ln(a) d-major
            pos_sb = pp.tile([128, NT, D], BF16)
            dx_sb = pp.tile([128, ND, K], BF16)
            dy_sb = pp.tile([128, ND, K], BF16)
            e_sb = pp.tile([128, NKC, D], BF16)
            # half-batch scratch ([128, 8, 256] = one token half). Multi-purpose;
            # phase ordering keeps uses disjoint (WAR tracked by the framework).
            a0_all = pp.tile([128, 8, D], BF16)        # a half0 / u staging
            a1_all = pp.tile([128, 8, D], BF16)        # a half1 / allreduced u
            uln_all = pp.tile([128, 8, D], F32)        # ln(u) / embed gathers
            lnt_all = pp.tile([128, 8, D], BF16)       # ln(a) chunks / square sink

            nc.sync.dma_start(pos_sb[:], pos_d.ap().rearrange("(j p) d -> p j d", p=128))
            nc.sync.dma_start(dx_sb[:], dx_d.ap().rearrange("(c p) k -> p c k", p=128))
            nc.sync.dma_start(dy_sb[:], dy_d.ap().rearrange("(c p) k -> p c k", p=128))
            nc.sync.dma_start(e_sb[:], e_d.ap().rearrange("(c p) d -> p c d", p=128))

            # ---- internal DRAM (collective staging, bf16) ----
            cc_in = [dpool.tile([8, 128, D], BF16, tag=f"cci{i}", name=f"cci{i}")
                     for i in range(2)]
            cc_out = [dpool.tile([8, 128, D], BF16, tag=f"cco{i}", name=f"cco{i}")
                      for i in range(2)]

            def stats_tail(sum_t, sq_t):
                """Scalar chain from per-chunk sums/sumsqs: (rstd, nmr) [128,8]."""
                negm = stp.tile([128, 8], F32, tag="st", name=_nm("st_"))
                nc.vector.tensor_scalar_mul(negm[:], sum_t[:], -1.0 / D)
                msq = stp.tile([128, 8], F32, tag="st", name=_nm("st_"))
                nc.vector.tensor_mul(msq[:], negm[:], negm[:])
                var = stp.tile([128, 8], F32, tag="st", name=_nm("st_"))
                nc.vector.scalar_tensor_tensor(
                    out=var[:], in0=sq_t[:], scalar=1.0 / D, in1=msq[:],
                    op0=OP.mult, op1=OP.subtract)
                sd = stp.tile([128, 8], F32, tag="st", name=_nm("st_"))
                nc.scalar.activation(sd[:], var[:], AF.Sqrt, bias=eps_p[:])
                rstd = stp.tile([128, 8], F32, tag="st", name=_nm("st_"))
                nc.vector.reciprocal(rstd[:], sd[:])
                nmr = stp.tile([128, 8], F32, tag="st", name=_nm("st_"))
                nc.vector.tensor_mul(nmr[:], negm[:], rstd[:])
                return rstd, nmr

            def sq_dve(src3d_ap, sq_t):
                """sumsq via DVE mul into lnt_all then DVE reduce (short chain)."""
                nc.vector.tensor_mul(lnt_all[:], src3d_ap, src3d_ap)
                nc.vector.tensor_reduce(sq_t[:], lnt_all[:], axis=AX.X, op=OP.add)

            def sq_gp(src3d_ap, sq_t):
                """sumsq via GpSimd mul (off critical path) + DVE reduce."""
                nc.gpsimd.tensor_mul(lnt_all[:], src3d_ap, src3d_ap)
                nc.vector.tensor_reduce(sq_t[:], lnt_all[:], axis=AX.X, op=OP.add)

            def apply_half(dst_tile, dst0, src_tile, src0, rstd, nmr):
                for j8 in range(8):
                    nc.scalar.activation(dst_tile[:, dst0 + j8],
                                         src_tile[:, src0 + j8], AF.Identity,
                                         bias=nmr[:, j8:j8 + 1],
                                         scale=rstd[:, j8:j8 + 1])

            def transpose_half(src_tile, sl, dst, c0):
                """Transpose 8 [128, 256] td chunks into dst[:, dc, c0:c0+1024]."""
                tpa = ptp.tile([128, TH], BF16, tag="pt", name=_nm("pt_"))
                tpb = ptp.tile([128, TH], BF16, tag="pt", name=_nm("pt_"))
                for j8 in range(8):
                    nc.tensor.transpose(tpa[:, j8 * 128:(j8 + 1) * 128],
                                        src_tile[:, sl + j8, 0:128], ident_bf[:])
                    nc.tensor.transpose(tpb[:, j8 * 128:(j8 + 1) * 128],
                                        src_tile[:, sl + j8, 128:256], ident_bf[:])
                nc.scalar.copy(dst[:, 0, c0:c0 + TH], tpa[:])
                nc.scalar.copy(dst[:, 1, c0:c0 + TH], tpb[:])

            # ================= embedding gather + LN; vp = ln + pos ==========
            idx = pp.tile([128, NT], I32)
            nc.sync.dma_start(idx[:], tok_d.ap().rearrange("(n p) -> p n", p=128))
            for th in range(2):
                h0 = th * 8
                sl = slice(h0, h0 + 8)
                for j8 in range(8):
                    nc.gpsimd.indirect_dma_start(
                        out=uln_all[:, j8], out_offset=None, in_=emb_d.ap(),
                        in_offset=bass.IndirectOffsetOnAxis(
                            ap=idx[:, h0 + j8:h0 + j8 + 1], axis=0),
                    )
                sum_t = stp.tile([128, 8], F32, tag="st", name=_nm("st_"))
                nc.vector.tensor_reduce(sum_t[:], uln_all[:], axis=AX.X, op=OP.add)
                sq_t = stp.tile([128, 8], F32, tag="st", name=_nm("st_"))
                sq_dve(uln_all[:], sq_t)
                rstd, nmr = stats_tail(sum_t, sq_t)
                apply_half(v_td, h0, uln_all, 0, rstd, nmr)
                nc.vector.tensor_add(vp_bf[:, sl], v_td[:, sl], pos_sb[:, sl])
                transpose_half(vp_bf, h0, vpT, th * TH)

            def phaseB(th):
                """x[:, th cols] = relu(vp @ Dx); RoPE -> q; build qtk."""
                c0 = th * TH

                def px_one(i):
                    px = pbp.tile([128, TH], F32, tag="pb", name=_nm("pb_"))
                    for dc in range(ND):
                        for ns in range(2):
                            nc.tensor.matmul(
                                px[:, ns * 512:(ns + 1) * 512],
                                dx_sb[:, dc, i * 128:(i + 1) * 128],
                                vpT[:, dc, c0 + ns * 512:c0 + (ns + 1) * 512],
                                start=(dc == 0), stop=(dc == ND - 1))
                    nc.scalar.activation(x_bf[:, i, c0:c0 + TH], px[:], AF.Relu)

                def rope_one(i):
                    cos_t = w4p.tile([128, TH], BF16, tag="w4", name=_nm("w4_"))
                    nc.sync.dma_start(cos_t[:], cos_d.ap()[i * 2 + th])
                    sin_t = w4p.tile([128, TH], BF16, tag="w4", name=_nm("w4_"))
                    nc.sync.dma_start(sin_t[:], sin_d.ap()[i * 2 + th])
                    xi = x_bf[:, i, c0:c0 + TH]
                    xj = x_bf[:, i + 4, c0:c0 + TH]
                    ma = w4p.tile([128, TH], BF16, tag="w4", name=_nm("w4_"))
                    nc.vector.tensor_mul(ma[:], xi, cos_t[:])
                    mb = w4p.tile([128, TH], BF16, tag="w4", name=_nm("w4_"))
                    nc.vector.tensor_mul(mb[:], xj, sin_t[:])
                    nc.vector.tensor_sub(qT[:, i, c0:c0 + TH], ma[:], mb[:])
                    nc.vector.tensor_mul(ma[:], xj, cos_t[:])
                    nc.vector.tensor_mul(mb[:], xi, sin_t[:])
                    nc.vector.tensor_add(qT[:, i + 4, c0:c0 + TH], ma[:], mb[:])

                # pair-interleaved so rope(i) can start while px continues
                px_one(0); px_one(4); rope_one(0)
                px_one(1); px_one(5); rope_one(1)
                px_one(2); px_one(6); rope_one(2)
                px_one(3); px_one(7); rope_one(3)
                for j in range(th * 8, th * 8 + 8):
                    tq = ptp.tile([128, K], BF16, tag="pt", name=_nm("pt_"))
                    for kc in range(NKC):
                        nc.tensor.transpose(tq[:, kc * 128:(kc + 1) * 128],
                                            qT[:, kc, j * 128:(j + 1) * 128],
                                            ident_bf[:])
                    nc.scalar.copy(qtk[:, j], tq[:])

            def phaseG(half):
                """G half-accumulation over token chunks; half 1 finalizes g_bf."""
                for kc in range(NKC):
                    pg = pap.tile([128, D], F32, tag="pa", name=_nm("pa_"))
                    for j in range(half * 8, half * 8 + 8):
                        nc.tensor.matmul(pg[:], qtk[:, j, kc * 128:(kc + 1) * 128],
                                         vp_bf[:, j],
                                         start=(j == half * 8), stop=(j == half * 8 + 7))
                    if half == 0:
                        nc.scalar.copy(g0_sb[:, kc], pg[:])
                    else:
                        nc.vector.tensor_add(g_bf[:, kc], g0_sb[:, kc], pg[:])

            def phaseCa_mm(half, dst_tile, sum_t):
                """a = q G matmuls for a token half; copies carry the sums."""
                h0 = half * 8
                for j8 in range(8):
                    j = h0 + j8
                    paa = pap.tile([128, D], F32, tag="pa", name=_nm("pa_"))
                    for kc in range(NKC):
                        nc.tensor.matmul(paa[:], qT[:, kc, j * 128:(j + 1) * 128],
                                         g_bf[:, kc],
                                         start=(kc == 0), stop=(kc == NKC - 1))
                    nc.scalar.activation(dst_tile[:, j8], paa[:], AF.Copy,
                                         accum_out=sum_t[:, j8:j8 + 1])

            def phaseCa_fin(half, src_tile, sum_t):
                """Batched LN of the a half -> lnA_dT."""
                sq_t = stp.tile([128, 8], F32, tag="st", name=_nm("st_"))
                sq_dve(src_tile[:], sq_t)
                rstd, nmr = stats_tail(sum_t, sq_t)
                apply_half(lnt_all, 0, src_tile, 0, rstd, nmr)
                transpose_half(lnt_all, 0, lnA_dT, half * TH)

            def phaseDy(th):
                """y = relu(lnA@Dy)*x into yt (aliases q's buffer)."""
                c0 = th * TH
                yt = qT
                for i in range(NKC):
                    py = pbp.tile([128, TH], F32, tag="pb", name=_nm("pb_"))
                    for dc in range(ND):
                        for ns in range(2):
                            nc.tensor.matmul(
                                py[:, ns * 512:(ns + 1) * 512],
                                dy_sb[:, dc, i * 128:(i + 1) * 128],
                                lnA_dT[:, dc, c0 + ns * 512:c0 + (ns + 1) * 512],
                                start=(dc == 0), stop=(dc == ND - 1))
                    nc.vector.scalar_tensor_tensor(
                        out=yt[:, i, c0:c0 + TH], in0=py[:], scalar=0.0,
                        in1=x_bf[:, i, c0:c0 + TH], op0=OP.max, op1=OP.mult)

            def phaseDu(th):
                """u = y@E (token-major); stage bf16 (a0_all) and AllReduce."""
                yt = qT
                for j8 in range(8):
                    j = th * 8 + j8
                    pu = pap.tile([128, D], F32, tag="pa", name=_nm("pa_"))
                    for i in range(NKC):
                        nc.tensor.matmul(pu[:], yt[:, i, j * 128:(j + 1) * 128],
                                         e_sb[:, i],
                                         start=(i == 0), stop=(i == NKC - 1))
                    nc.scalar.copy(a0_all[:, j8], pu[:])
                nc.sync.dma_start(
                    cc_in[th][:].rearrange("j p d -> p j d"), a0_all[:])
                nc.gpsimd.collective_compute(
                    "AllReduce", OP.add, replica_groups=groups,
                    ins=[cc_in[th][:].opt()], outs=[cc_out[th][:].opt()])

            def phaseE_chain(th, layer):
                """w = v+pos+ln(u); v = ln(w); vp = v+pos' (no PE work)."""
                last = layer == N_LAYERS - 1
                h0 = th * 8
                sl = slice(h0, h0 + 8)
                # +pos is independent of the collective -- issue first, hidden
                nc.vector.tensor_add(v_td[:, sl], v_td[:, sl], pos_sb[:, sl])
                nc.sync.dma_start(a1_all[:],
                                  cc_out[th][:].rearrange("j p d -> p j d"))
                sum_t = stp.tile([128, 8], F32, tag="st", name=_nm("st_"))
                nc.vector.tensor_reduce(sum_t[:], a1_all[:], axis=AX.X, op=OP.add)
                sq_t = stp.tile([128, 8], F32, tag="st", name=_nm("st_"))
                sq_gp(a1_all[:], sq_t)
                rstd, nmr = stats_tail(sum_t, sq_t)
                apply_half(uln_all, 0, a1_all, 0, rstd, nmr)
                nc.vector.tensor_add(v_td[:, sl], v_td[:, sl], uln_all[:])
                sum_w = stp.tile([128, 8], F32, tag="st", name=_nm("st_"))
                nc.vector.tensor_reduce(sum_w[:], v_td[:, sl], axis=AX.X, op=OP.add)
                sq_w = stp.tile([128, 8], F32, tag="st", name=_nm("st_"))
                sq_gp(v_td[:, sl], sq_w)
                rstd_w, nmr_w = stats_tail(sum_w, sq_w)
                apply_half(v_td, h0, v_td, h0, rstd_w, nmr_w)
                if not last:
                    nc.vector.tensor_add(vp_bf[:, sl], v_td[:, sl], pos_sb[:, sl])
                else:
                    nc.scalar.copy(vp_bf[:, sl], v_td[:, sl])

            def phaseE_tp(th):
                transpose_half(vp_bf, th * 8, vpT, th * TH)

            def readout_half(th):
                """logitsT[:, th cols] = (v @ readout)^T for the token half."""
                nvb = (VS + 127) // 128
                for vb in range(nvb):
                    m = min(128, VS - vb * 128)
                    rot = rop.tile([128, ND, 128], BF16, tag="ro", name=_nm("ro_"))
                    for dc in range(ND):
                        nc.sync.dma_start(
                            rot[:, dc, :m],
                            ro_d.ap()[dc * 128:(dc + 1) * 128,
                                      vb * 128:vb * 128 + m])
                    pl = pbp.tile([128, TH], F32, tag="pb", name=_nm("pb_"))
                    for dc in range(ND):
                        for ns in range(2):
                            nc.tensor.matmul(
                                pl[:m, ns * 512:(ns + 1) * 512],
                                rot[:, dc, :m],
                                vpT[:, dc, th * TH + ns * 512:
                                    th * TH + (ns + 1) * 512],
                                start=(dc == 0), stop=(dc == ND - 1))
                    lo = lop.tile([128, TH], BF16, tag="lo", name=_nm("lo_"))
                    if vb % 2 == 0:
                        nc.vector.tensor_copy(lo[:m], pl[:m])
                    else:
                        nc.scalar.copy(lo[:m], pl[:m])
                    nc.sync.dma_start(
                        out_d.ap()[vb * 128:vb * 128 + m, th * TH:(th + 1) * TH],
                        lo[:m])

            # ================================ layers ================================
            phaseB(0)
            phaseB(1)
            phaseG(0)
            phaseG(1)
            for layer in range(N_LAYERS):
                last = layer == N_LAYERS - 1
                with nc.named_scope(f"L{layer}"):
                    sum_a0 = stp.tile([128, 8], F32, tag="st", name=_nm("st_"))
                    sum_a1 = stp.tile([128, 8], F32, tag="st", name=_nm("st_"))
                    phaseCa_mm(0, a0_all, sum_a0)
                    phaseCa_mm(1, a1_all, sum_a1)   # PE covers half0's LN stats
                    phaseCa_fin(0, a0_all, sum_a0)
                    phaseDy(0)
                    phaseCa_fin(1, a1_all, sum_a1)  # stats overlap Dy(0)
                    phaseDu(0)          # cc0 in flight...
                    phaseE_chain(0, layer)   # ...its chain overlaps D(th1)
                    phaseDy(1)
                    phaseDu(1)          # cc1 in flight...
                    phaseE_tp(0)
                    if not last:
                        phaseB(0)       # ...covered by B/G(th0)
                        phaseG(0)
                        phaseE_chain(1, layer)
                        phaseE_tp(1)
                        phaseB(1)
                        phaseG(1)
                    else:
                        if DO_READOUT:
                            readout_half(0)
                        phaseE_chain(1, layer)
                        phaseE_tp(1)
                        if DO_READOUT:
                            readout_half(1)

    nc.compile()
    return nc


_NC_CACHE = None


def _get_nc():
    global _NC_CACHE
    if _NC_CACHE is None:
        nc = bacc.Bacc("TRN2", target_bir_lowering=False, debug=False, num_devices=8)
        _NC_CACHE = build(nc)
    return _NC_CACHE


def _rope_tables():
    # match the jax reference: float32 angle computation, then bf16 cast
    import ml_dtypes
    inv_freq = (1.0 / (10000.0 ** (np.arange(0, K, 2, dtype=np.float32)
                                   / np.float32(K)))).astype(np.float32)
    t = np.arange(T, dtype=np.float32)
    freqs = (t[:, None] * inv_freq[None, :]).astype(np.float32)  # [T, K/2]
    cos = np.cos(freqs).astype(np.float32)
    sin = np.sin(freqs).astype(np.float32)
    # [K/2, T] -> [4, 128, 2, 1024] -> [8, 128, 1024] with index i*2+th
    def pack(a):
        aT = np.ascontiguousarray(a.T).reshape(4, 128, 2, TH)
        return np.ascontiguousarray(
            aT.transpose(0, 2, 1, 3).reshape(8, 128, TH)).astype(ml_dtypes.bfloat16)
    return pack(cos), pack(sin)


def kernel(input_, emb, pos, Dx, Dy, E, readout):
    import ml_dtypes
    BF = ml_dtypes.bfloat16
    input_ = np.asarray(input_)
    emb = np.ascontiguousarray(np.asarray(emb, dtype=np.float32))
    pos = np.ascontiguousarray(np.asarray(pos, dtype=np.float32))
    Dx = np.asarray(Dx, dtype=np.float32)
    Dy = np.asarray(Dy, dtype=np.float32)
    E = np.asarray(E, dtype=np.float32)
    readout = np.asarray(readout, dtype=np.float32)

    nc = _get_nc()
    cosb, sinb = _rope_tables()
    ro_bf = readout.astype(BF)

    in_maps = []
    for c in range(8):
        b, h = divmod(c, 4)
        in_maps.append({
            "tok": np.ascontiguousarray(input_[b].astype(np.int32)),
            "emb": emb,
            "posb": np.ascontiguousarray(pos.astype(BF)),
            "dxb": np.ascontiguousarray(Dx[h].astype(BF)),
            "dyb": np.ascontiguousarray(Dy[h].astype(BF)),
            "eb": np.ascontiguousarray(E[h * K:(h + 1) * K].astype(BF)),
            "rob": np.ascontiguousarray(ro_bf[:, h * VS:(h + 1) * VS]),
            "cosb": cosb,
            "sinb": sinb,
        })
    trace = os.environ.get("KRN_TRACE", "0") == "1"
    res = run_bass_kernel_spmd(nc, in_maps, list(range(8)), trace=trace)
    out = np.empty((B, T, V), dtype=np.float32)
    for c in range(8):
        b, h = divmod(c, 4)
        out[b, :, h * VS:(h + 1) * VS] = res.results[c]["logitsT"].astype(np.float32).T
    kernel._last_results = res
    return out


# revision 19
# speedup vs baseline: 1.1287x; 1.1287x over previous
"""Trainium2 Bass kernel for nn_BDH_4406636445711 (dense transformer).

Sharding: 8 cores = data-parallel over B(2) x tensor-parallel over H(4).
Core c handles (b = c//4, h = c%4): its head's Dx/Dy slices, E rows, and a
V/4 shard of the readout. Per layer the y@E partial is AllReduced (bf16)
within each b-group of 4 cores. The host stitches the 8 per-core [VS, T]
logit shards (bf16 on device, cast to fp32 host-side) into [B, T, V].

Key algebraic optimization vs the naive graph: scores = q @ q^T is only
ever used for a = scores @ v, so we compute a = q @ (q^T v) instead --
G = q^T v is [K, D]; ~5x fewer PE cycles than materializing [T, T] scores.

Layouts: v lives token-major ("td": [128 tok part, 16 chunk, 256 d]) so
every LayerNorm is a free-dim reduction. x/q live kT; q is additionally
transposed to tk for the G matmul. All matmul operands are bf16 (full PE
rate); accumulation and LN stats stay fp32. ACT only ever needs the
{relu, copy, sqrt, square, identity} table -- no table reloads.

Scheduling notes:
- Both a-halves' matmuls are issued back-to-back so the first half's
  batched LN statistics overlap the second half's matmuls; the a-sums
  ride for free on the PSUM->SBUF copies via the ACT accumulator.
- v_td holds ln(w) WITHOUT pos; the +pos happens (a) fused into the
  vp_bf cast (one DVE add, bf16 out) and (b) at the top of the next
  E chain where it is latency-hidden. This shortens the E tail that
  gates the next phase-B matmuls.
- The layer is software-pipelined around the two AllReduces: E(th0)'s
  stat chain overlaps D(th1)'s matmuls; the next layer's B/G halves
  cover the second collective. The readout runs as two column-half
  passes so the first pass overlaps the last layer's E(th1) chain.
"""

import os
import sys

sys.path.insert(0, "/opt/trn_rl_repo")

import numpy as np

import concourse.bass as bass
import concourse.tile as tile
from concourse import bacc, mybir
from concourse.bass_utils import run_bass_kernel_spmd
from concourse.masks import make_identity
from concourse import library_config

F32 = mybir.dt.float32
BF16 = mybir.dt.bfloat16
I32 = mybir.dt.int32
AF = mybir.ActivationFunctionType
OP = mybir.AluOpType
AX = mybir.AxisListType

B, T, H, D, K, V, L = 2, 2048, 4, 256, 1024, 32000, 6
VS = V // 4          # vocab shard per core within a b-group
EPS = 1e-5
NT = T // 128        # 16 token chunks
NKC = K // 128       # 8 k chunks
ND = D // 128        # 2 d chunks
TH = T // 2          # 1024

N_LAYERS = int(os.environ.get("KRN_LAYERS", str(L)))
DO_READOUT = os.environ.get("KRN_READOUT", "1") == "1"


def build(nc):
    # ---- DRAM parameters (per core) ----
    tok_d = nc.dram_tensor("tok", [T], I32, kind="ExternalInput")
    emb_d = nc.dram_tensor("emb", [V, D], F32, kind="ExternalInput")
    pos_d = nc.dram_tensor("posb", [T, D], BF16, kind="ExternalInput")
    dx_d = nc.dram_tensor("dxb", [D, K], BF16, kind="ExternalInput")
    dy_d = nc.dram_tensor("dyb", [D, K], BF16, kind="ExternalInput")
    e_d = nc.dram_tensor("eb", [K, D], BF16, kind="ExternalInput")
    ro_d = nc.dram_tensor("rob", [D, VS], BF16, kind="ExternalInput")
    cos_d = nc.dram_tensor("cosb", [8, 128, TH], BF16, kind="ExternalInput")
    sin_d = nc.dram_tensor("sinb", [8, 128, TH], BF16, kind="ExternalInput")
    out_d = nc.dram_tensor("logitsT", [VS, T], BF16, kind="ExternalOutput")

    groups = [[0, 1, 2, 3], [4, 5, 6, 7]]

    with tile.TileContext(nc) as tc:
        with (
            nc.allow_low_precision(reason="bf16 matmul path is intentional"),
            tc.tile_pool(name="persist", bufs=1) as pp,
            tc.tile_pool(name="w4", bufs=4) as w4p,     # [128,1024] bf16 rope
            tc.tile_pool(name="stats", bufs=24) as stp, # [128,8] f32
            tc.tile_pool(name="rop", bufs=3) as rop,    # readout weights
            tc.tile_pool(name="lop", bufs=3) as lop,    # logit staging
            tc.tile_pool(name="pb", bufs=2, space="PSUM") as pbp,   # [128,1024] f32
            tc.tile_pool(name="pa", bufs=2, space="PSUM") as pap,   # [128,256] f32
            tc.tile_pool(name="pt", bufs=2, space="PSUM") as ptp,   # [128,1024] bf16
            tc.tile_pool(name="dram", bufs=1, space="DRAM") as dpool,
        ):
            _ctr = [0]

            def _nm(p):
                _ctr[0] += 1
                return f"{p}{_ctr[0]}"

            # ---- constants ----
            ident_f = pp.tile([128, 128], F32)
            make_identity(nc, ident_f[:])
            ident_bf = pp.tile([128, 128], BF16)
            nc.vector.tensor_copy(ident_bf[:], ident_f[:])
            eps_p = pp.tile([128, 1], F32)
            nc.vector.memset(eps_p[:], EPS)
            nc.gpsimd.load_library(library_config.attn)

            # ---- persistent tensors ----
            v_td = pp.tile([128, NT, D], F32)          # ln(w), token-major (no pos)
            vp_bf = pp.tile([128, NT, D], BF16)        # v + pos, bf16
            vpT = pp.tile([128, ND, T], BF16)          # (v+pos) transposed
            qT = pp.tile([128, NKC, T], BF16)          # q k-major; reused as yt
            qtk = pp.tile([128, NT, K], BF16)          # q token-major
            x_bf = pp.tile([128, NKC, T], BF16)        # relu(v@Dx), k-major
            g0_sb = pp.tile([128, NKC, D], BF16)       # G partial (th0 tokens)
            g_bf = pp.tile([128, NKC, D], BF16)        # G = q^T (v+pos), full
            lnA_dT = pp.tile([128, ND, T], BF16)       # ln(a) d-major
            pos_sb = pp.tile([128, NT, D], BF16)
            dx_sb = pp.tile([128, ND, K], BF16)
            dy_sb = pp.tile([128, ND, K], BF16)
            e_sb = pp.tile([128, NKC, D], BF16)
            # half-batch scratch ([128, 8, 256] = one token half). Multi-purpose;
            # phase ordering keeps uses disjoint (WAR tracked by the framework).
            a0_all = pp.tile([128, 8, D], BF16)        # a half0 / u staging
            a1_all = pp.tile([128, 8, D], BF16)        # a half1 / allreduced u
            uln_all = pp.tile([128, 8, D], F32)        # ln(u) / embed gathers
            lnt_all = pp.tile([128, 8, D], BF16)       # ln(a) chunks / square sink

            nc.sync.dma_start(pos_sb[:], pos_d.ap().rearrange("(j p) d -> p j d", p=128))
            nc.sync.dma_start(dx_sb[:], dx_d.ap().rearrange("(c p) k -> p c k", p=128))
            nc.sync.dma_start(dy_sb[:], dy_d.ap().rearrange("(c p) k -> p c k", p=128))
            nc.sync.dma_start(e_sb[:], e_d.ap().rearrange("(c p) d -> p c d", p=128))

            # ---- internal DRAM (collective staging, bf16) ----
            cc_in = [dpool.tile([8, 128, D], BF16, tag=f"cci{i}", name=f"cci{i}")
                     for i in range(2)]
            cc_out = [dpool.tile([8, 128, D], BF16, tag=f"cco{i}", name=f"cco{i}")
                      for i in range(2)]

            def stats_tail(sum_t, sq_t):
                """Scalar chain from per-chunk sums/sumsqs: (rstd, nmr) [128,8]."""
                negm = stp.tile([128, 8], F32, tag="st", name=_nm("st_"))
                nc.vector.tensor_scalar_mul(negm[:], sum_t[:], -1.0 / D)
                msq = stp.tile([128, 8], F32, tag="st", name=_nm("st_"))
                nc.vector.tensor_mul(msq[:], negm[:], negm[:])
                var = stp.tile([128, 8], F32, tag="st", name=_nm("st_"))
                nc.vector.scalar_tensor_tensor(
                    out=var[:], in0=sq_t[:], scalar=1.0 / D, in1=msq[:],
                    op0=OP.mult, op1=OP.subtract)
                sd = stp.tile([128, 8], F32, tag="st", name=_nm("st_"))
                nc.scalar.activation(sd[:], var[:], AF.Sqrt, bias=eps_p[:])
                rstd = stp.tile([128, 8], F32, tag="st", name=_nm("st_"))
                nc.vector.reciprocal(rstd[:], sd[:])
                nmr = stp.tile([128, 8], F32, tag="st", name=_nm("st_"))
                nc.vector.tensor_mul(nmr[:], negm[:], rstd[:])
                return rstd, nmr

            def sq_dve(src3d_ap, sq_t):
                """sumsq via DVE mul into lnt_all then DVE reduce (short chain)."""
                nc.vector.tensor_mul(lnt_all[:], src3d_ap, src3d_ap)
                nc.vector.tensor_reduce(sq_t[:], lnt_all[:], axis=AX.X, op=OP.add)

            def apply_half(dst_tile, dst0, src_tile, src0, rstd, nmr):
                for j8 in range(8):
                    nc.scalar.activation(dst_tile[:, dst0 + j8],
                                         src_tile[:, src0 + j8], AF.Identity,
                                         bias=nmr[:, j8:j8 + 1],
                                         scale=rstd[:, j8:j8 + 1])

            def transpose_half(src_tile, sl, dst, c0):
                """Transpose 8 [128, 256] td chunks into dst[:, dc, c0:c0+1024]."""
                tpa = ptp.tile([128, TH], BF16, tag="pt", name=_nm("pt_"))
                tpb = ptp.tile([128, TH], BF16, tag="pt", name=_nm("pt_"))
                for j8 in range(8):
                    nc.tensor.transpose(tpa[:, j8 * 128:(j8 + 1) * 128],
                                        src_tile[:, sl + j8, 0:128], ident_bf[:])
                    nc.tensor.transpose(tpb[:, j8 * 128:(j8 + 1) * 128],
                                        src_tile[:, sl + j8, 128:256], ident_bf[:])
                nc.scalar.copy(dst[:, 0, c0:c0 + TH], tpa[:])
                nc.scalar.copy(dst[:, 1, c0:c0 + TH], tpb[:])

            # ================= embedding gather + LN; vp = ln + pos ==========
            idx = pp.tile([128, NT], I32)
            nc.sync.dma_start(idx[:], tok_d.ap().rearrange("(n p) -> p n", p=128))
            for th in range(2):
                h0 = th * 8
                sl = slice(h0, h0 + 8)
                for j8 in range(8):
                    nc.gpsimd.indirect_dma_start(
                        out=uln_all[:, j8], out_offset=None, in_=emb_d.ap(),
                        in_offset=bass.IndirectOffsetOnAxis(
                            ap=idx[:, h0 + j8:h0 + j8 + 1], axis=0),
                    )
                sum_t = stp.tile([128, 8], F32, tag="st", name=_nm("st_"))
                nc.vector.tensor_reduce(sum_t[:], uln_all[:], axis=AX.X, op=OP.add)
                sq_t = stp.tile([128, 8], F32, tag="st", name=_nm("st_"))
                sq_dve(uln_all[:], sq_t)
                rstd, nmr = stats_tail(sum_t, sq_t)
                apply_half(v_td, h0, uln_all, 0, rstd, nmr)
                nc.vector.tensor_add(vp_bf[:, sl], v_td[:, sl], pos_sb[:, sl])
                transpose_half(vp_bf, h0, vpT, th * TH)

            def phaseB(th):
                """x[:, th cols] = relu(vp @ Dx); RoPE -> q; build qtk."""
                c0 = th * TH

                def px_one(i):
                    px = pbp.tile([128, TH], F32, tag="pb", name=_nm("pb_"))
                    for dc in range(ND):
                        for ns in range(2):
                            nc.tensor.matmul(
                                px[:, ns * 512:(ns + 1) * 512],
                                dx_sb[:, dc, i * 128:(i + 1) * 128],
                                vpT[:, dc, c0 + ns * 512:c0 + (ns + 1) * 512],
                                start=(dc == 0), stop=(dc == ND - 1))
                    nc.scalar.activation(x_bf[:, i, c0:c0 + TH], px[:], AF.Relu)

                def rope_one(i):
                    cos_t = w4p.tile([128, TH], BF16, tag="w4", name=_nm("w4_"))
                    nc.sync.dma_start(cos_t[:], cos_d.ap()[i * 2 + th])
                    sin_t = w4p.tile([128, TH], BF16, tag="w4", name=_nm("w4_"))
                    nc.sync.dma_start(sin_t[:], sin_d.ap()[i * 2 + th])
                    xi = x_bf[:, i, c0:c0 + TH]
                    xj = x_bf[:, i + 4, c0:c0 + TH]
                    ma = w4p.tile([128, TH], BF16, tag="w4", name=_nm("w4_"))
                    nc.vector.tensor_mul(ma[:], xi, cos_t[:])
                    mb = w4p.tile([128, TH], BF16, tag="w4", name=_nm("w4_"))
                    nc.vector.tensor_mul(mb[:], xj, sin_t[:])
                    nc.vector.tensor_sub(qT[:, i, c0:c0 + TH], ma[:], mb[:])
                    nc.vector.tensor_mul(ma[:], xj, cos_t[:])
                    nc.vector.tensor_mul(mb[:], xi, sin_t[:])
                    nc.vector.tensor_add(qT[:, i + 4, c0:c0 + TH], ma[:], mb[:])

                # pair-interleaved so rope(i) can start while px continues
                px_one(0); px_one(4); rope_one(0)
                px_one(1); px_one(5); rope_one(1)
                px_one(2); px_one(6); rope_one(2)
                px_one(3); px_one(7); rope_one(3)
                for j in range(th * 8, th * 8 + 8):
                    tq = ptp.tile([128, K], BF16, tag="pt", name=_nm("pt_"))
                    for kc in range(NKC):
                        nc.tensor.transpose(tq[:, kc * 128:(kc + 1) * 128],
                                            qT[:, kc, j * 128:(j + 1) * 128],
                                            ident_bf[:])
                    nc.vector.tensor_copy(qtk[:, j], tq[:])

            def phaseG(half):
                """G half-accumulation over token chunks; half 1 finalizes g_bf."""
                for kc in range(NKC):
                    pg = pap.tile([128, D], F32, tag="pa", name=_nm("pa_"))
                    for j in range(half * 8, half * 8 + 8):
                        nc.tensor.matmul(pg[:], qtk[:, j, kc * 128:(kc + 1) * 128],
                                         vp_bf[:, j],
                                         start=(j == half * 8), stop=(j == half * 8 + 7))
                    if half == 0:
                        nc.scalar.copy(g0_sb[:, kc], pg[:])
                    else:
                        nc.vector.tensor_add(g_bf[:, kc], g0_sb[:, kc], pg[:])

            def phaseCa_mm(half, dst_tile, sum_t):
                """a = q G matmuls for a token half; copies carry the sums."""
                h0 = half * 8
                for j8 in range(8):
                    j = h0 + j8
                    paa = pap.tile([128, D], F32, tag="pa", name=_nm("pa_"))
                    for kc in range(NKC):
                        nc.tensor.matmul(paa[:], qT[:, kc, j * 128:(j + 1) * 128],
                                         g_bf[:, kc],
                                         start=(kc == 0), stop=(kc == NKC - 1))
                    nc.scalar.activation(dst_tile[:, j8], paa[:], AF.Copy,
                                         accum_out=sum_t[:, j8:j8 + 1])

            def phaseCa_fin(half, src_tile, sum_t):
                """Batched LN of the a half -> lnA_dT."""
                sq_t = stp.tile([128, 8], F32, tag="st", name=_nm("st_"))
                sq_dve(src_tile[:], sq_t)
                rstd, nmr = stats_tail(sum_t, sq_t)
                apply_half(lnt_all, 0, src_tile, 0, rstd, nmr)
                transpose_half(lnt_all, 0, lnA_dT, half * TH)

            def phaseDy(th):
                """y = relu(lnA@Dy)*x into yt (aliases q's buffer)."""
                c0 = th * TH
                yt = qT
                for i in range(NKC):
                    py = pbp.tile([128, TH], F32, tag="pb", name=_nm("pb_"))
                    for dc in range(ND):
                        for ns in range(2):
                            nc.tensor.matmul(
                                py[:, ns * 512:(ns + 1) * 512],
                                dy_sb[:, dc, i * 128:(i + 1) * 128],
                                lnA_dT[:, dc, c0 + ns * 512:c0 + (ns + 1) * 512],
                                start=(dc == 0), stop=(dc == ND - 1))
                    nc.vector.scalar_tensor_tensor(
                        out=yt[:, i, c0:c0 + TH], in0=py[:], scalar=0.0,
                        in1=x_bf[:, i, c0:c0 + TH], op0=OP.max, op1=OP.mult)

            def phaseDu(th):
                """u = y@E (token-major); stage bf16 (a0_all) and AllReduce."""
                yt = qT
                for j8 in range(8):
                    j = th * 8 + j8
                    pu = pap.tile([128, D], F32, tag="pa", name=_nm("pa_"))
                    for i in range(NKC):
                        nc.tensor.matmul(pu[:], yt[:, i, j * 128:(j + 1) * 128],
                                         e_sb[:, i],
                                         start=(i == 0), stop=(i == NKC - 1))
                    nc.scalar.copy(a0_all[:, j8], pu[:])
                nc.sync.dma_start(
                    cc_in[th][:].rearrange("j p d -> p j d"), a0_all[:])
                nc.gpsimd.collective_compute(
                    "AllReduce", OP.add, replica_groups=groups,
                    ins=[cc_in[th][:].opt()], outs=[cc_out[th][:].opt()])

            def phaseE_pre(th, ubuf):
                """+pos (collective-independent) and the u DMA; never blocks
                the DVE queue on the collective."""
                sl = slice(th * 8, th * 8 + 8)
                nc.vector.tensor_add(v_td[:, sl], v_td[:, sl], pos_sb[:, sl])
                nc.sync.dma_start(ubuf[:],
                                  cc_out[th][:].rearrange("j p d -> p j d"))

            def phaseE_stats(th, layer, ubuf):
                """w = (v+pos)+ln(u); v = ln(w); vp = v+pos' (no PE work)."""
                last = layer == N_LAYERS - 1
                h0 = th * 8
                sl = slice(h0, h0 + 8)
                nc.vector.tensor_mul(lnt_all[:], ubuf[:], ubuf[:])
                sum_t = stp.tile([128, 8], F32, tag="st", name=_nm("st_"))
                nc.vector.tensor_reduce(sum_t[:], ubuf[:], axis=AX.X, op=OP.add)
                sq_t = stp.tile([128, 8], F32, tag="st", name=_nm("st_"))
                nc.vector.tensor_reduce(sq_t[:], lnt_all[:], axis=AX.X, op=OP.add)
                rstd, nmr = stats_tail(sum_t, sq_t)
                apply_half(uln_all, 0, ubuf, 0, rstd, nmr)
                nc.vector.tensor_add(v_td[:, sl], v_td[:, sl], uln_all[:])
                nc.vector.tensor_mul(lnt_all[:], v_td[:, sl], v_td[:, sl])
                sum_w = stp.tile([128, 8], F32, tag="st", name=_nm("st_"))
                nc.vector.tensor_reduce(sum_w[:], v_td[:, sl], axis=AX.X, op=OP.add)
                sq_w = stp.tile([128, 8], F32, tag="st", name=_nm("st_"))
                nc.vector.tensor_reduce(sq_w[:], lnt_all[:], axis=AX.X, op=OP.add)
                rstd_w, nmr_w = stats_tail(sum_w, sq_w)
                apply_half(v_td, h0, v_td, h0, rstd_w, nmr_w)
                if not last:
                    nc.vector.tensor_add(vp_bf[:, sl], v_td[:, sl], pos_sb[:, sl])
                else:
                    nc.scalar.copy(vp_bf[:, sl], v_td[:, sl])

            def phaseE_tp(th):
                transpose_half(vp_bf, th * 8, vpT, th * TH)

            def readout_half(th):
                """logitsT[:, th cols] = (v @ readout)^T for the token half."""
                nvb = (VS + 127) // 128
                for vb in range(nvb):
                    m = min(128, VS - vb * 128)
                    rot = rop.tile([128, ND, 128], BF16, tag="ro", name=_nm("ro_"))
                    for dc in range(ND):
                        nc.sync.dma_start(
                            rot[:, dc, :m],
                            ro_d.ap()[dc * 128:(dc + 1) * 128,
                                      vb * 128:vb * 128 + m])
                    pl = pbp.tile([128, TH], F32, tag="pb", name=_nm("pb_"))
                    for dc in range(ND):
                        for ns in range(2):
                            nc.tensor.matmul(
                                pl[:m, ns * 512:(ns + 1) * 512],
                                rot[:, dc, :m],
                                vpT[:, dc, th * TH + ns * 512:
                                    th * TH + (ns + 1) * 512],
                                start=(dc == 0), stop=(dc == ND - 1))
                    lo = lop.tile([128, TH], BF16, tag="lo", name=_nm("lo_"))
                    if vb % 2 == 0:
                        nc.vector.tensor_copy(lo[:m], pl[:m])
                    else:
                        nc.scalar.copy(lo[:m], pl[:m])
                    nc.sync.dma_start(
                        out_d.ap()[vb * 128:vb * 128 + m, th * TH:(th + 1) * TH],
                        lo[:m])

            # ================================ layers ================================
            phaseB(0)
            phaseB(1)
            phaseG(0)
            phaseG(1)
            for layer in range(N_LAYERS):
                last = layer == N_LAYERS - 1
                with nc.named_scope(f"L{layer}"):
                    sum_a0 = stp.tile([128, 8], F32, tag="st", name=_nm("st_"))
                    sum_a1 = stp.tile([128, 8], F32, tag="st", name=_nm("st_"))
                    phaseCa_mm(0, a0_all, sum_a0)
                    phaseCa_mm(1, a1_all, sum_a1)   # PE covers half0's LN stats
                    phaseCa_fin(0, a0_all, sum_a0)
                    phaseDy(0)
                    phaseCa_fin(1, a1_all, sum_a1)  # stats overlap Dy(0)
                    phaseDu(0)              # cc0 in flight...
                    phaseE_pre(0, a1_all)   # u0 lands in a1_all
                    phaseDy(1)
                    phaseDu(1)              # cc1 in flight...
                    phaseE_stats(0, layer, a1_all)
                    phaseE_pre(1, a0_all)   # u1 lands in a0_all
                    phaseE_tp(0)
                    if not last:
                        phaseB(0)           # B/G(th0) cover cc1's flight
                        phaseG(0)
                        phaseE_stats(1, layer, a0_all)
                        phaseE_tp(1)
                        phaseB(1)
                        phaseG(1)
                    else:
                        if DO_READOUT:
                            readout_half(0)
                        phaseE_stats(1, layer, a0_all)
                        phaseE_tp(1)
                        if DO_READOUT:
                            readout_half(1)

    nc.compile()
    return nc


_NC_CACHE = None


def _get_nc():
    global _NC_CACHE
    if _NC_CACHE is None:
        nc = bacc.Bacc("TRN2", target_bir_lowering=False, debug=False, num_devices=8)
        _NC_CACHE = build(nc)
    return _NC_CACHE


def _rope_tables():
    # match the jax reference: float32 angle computation, then bf16 cast
    import ml_dtypes
    inv_freq = (1.0 / (10000.0 ** (np.arange(0, K, 2, dtype=np.float32)
                                   / np.float32(K)))).astype(np.float32)
    t = np.arange(T, dtype=np.float32)
    freqs = (t[:, None] * inv_freq[None, :]).astype(np.float32)  # [T, K/2]
    cos = np.cos(freqs).astype(np.float32)
    sin = np.sin(freqs).astype(np.float32)
    # [K/2, T] -> [4, 128, 2, 1024] -> [8, 128, 1024] with index i*2+th
    def pack(a):
        aT = np.ascontiguousarray(a.T).reshape(4, 128, 2, TH)
        return np.ascontiguousarray(
            aT.transpose(0, 2, 1, 3).reshape(8, 128, TH)).astype(ml_dtypes.bfloat16)
    return pack(cos), pack(sin)


def kernel(input_, emb, pos, Dx, Dy, E, readout):
    import ml_dtypes
    BF = ml_dtypes.bfloat16
    input_ = np.asarray(input_)
    emb = np.ascontiguousarray(np.asarray(emb, dtype=np.float32))
    pos = np.ascontiguousarray(np.asarray(pos, dtype=np.float32))
    Dx = np.asarray(Dx, dtype=np.float32)
    Dy = np.asarray(Dy, dtype=np.float32)
    E = np.asarray(E, dtype=np.float32)
    readout = np.asarray(readout, dtype=np.float32)

    nc = _get_nc()
    cosb, sinb = _rope_tables()
    ro_bf = readout.astype(BF)

    in_maps = []
    for c in range(8):
        b, h = divmod(c, 4)
        in_maps.append({
            "tok": np.ascontiguousarray(input_[b].astype(np.int32)),
            "emb": emb,
            "posb": np.ascontiguousarray(pos.astype(BF)),
            "dxb": np.ascontiguousarray(Dx[h].astype(BF)),
            "dyb": np.ascontiguousarray(Dy[h].astype(BF)),
            "eb": np.ascontiguousarray(E[h * K:(h + 1) * K].astype(BF)),
            "rob": np.ascontiguousarray(ro_bf[:, h * VS:(h + 1) * VS]),
            "cosb": cosb,
            "sinb": sinb,
        })
    trace = os.environ.get("KRN_TRACE", "0") == "1"
    res = run_bass_kernel_spmd(nc, in_maps, list(range(8)), trace=trace)
    out = np.empty((B, T, V), dtype=np.float32)
    for c in range(8):
        b, h = divmod(c, 4)
        out[b, :, h * VS:(h + 1) * VS] = res.results[c]["logitsT"].astype(np.float32).T
    kernel._last_results = res
    return out


# revision 20
# speedup vs baseline: 1.1406x; 1.0105x over previous
"""Trainium2 Bass kernel for nn_BDH_4406636445711 (dense transformer).

Sharding: 8 cores = data-parallel over B(2) x tensor-parallel over H(4).
Core c handles (b = c//4, h = c%4): its head's Dx/Dy slices, E rows, and a
V/4 shard of the readout. Per layer the y@E partial is AllReduced (bf16)
within each b-group of 4 cores. The host stitches the 8 per-core [VS, T]
logit shards (bf16 on device, cast to fp32 host-side) into [B, T, V].

Key algebraic optimization vs the naive graph: scores = q @ q^T is only
ever used for a = scores @ v, so we compute a = q @ (q^T v) instead --
G = q^T v is [K, D]; ~5x fewer PE cycles than materializing [T, T] scores.

Layouts: v lives token-major ("td": [128 tok part, 16 chunk, 256 d]) so
every LayerNorm is a free-dim reduction. x/q live kT; q is additionally
transposed to tk for the G matmul. All matmul operands are bf16 (full PE
rate); accumulation and LN stats stay fp32. ACT only ever needs the
{relu, copy, sqrt, square, identity} table -- no table reloads.

Scheduling notes:
- Both a-halves' matmuls are issued back-to-back so the first half's
  batched LN statistics overlap the second half's matmuls; the a-sums
  ride for free on the PSUM->SBUF copies via the ACT accumulator.
- v_td holds ln(w) WITHOUT pos; the +pos happens (a) fused into the
  vp_bf cast (one DVE add, bf16 out) and (b) at the top of the next
  E chain where it is latency-hidden. This shortens the E tail that
  gates the next phase-B matmuls.
- The layer is software-pipelined around the two AllReduces: E(th0)'s
  stat chain overlaps D(th1)'s matmuls; the next layer's B/G halves
  cover the second collective. The readout runs as two column-half
  passes so the first pass overlaps the last layer's E(th1) chain.
"""

import os
import sys

sys.path.insert(0, "/opt/trn_rl_repo")

import numpy as np

import concourse.bass as bass
import concourse.tile as tile
from concourse import bacc, mybir
from concourse.bass_utils import run_bass_kernel_spmd
from concourse.masks import make_identity
from concourse import library_config

F32 = mybir.dt.float32
BF16 = mybir.dt.bfloat16
I32 = mybir.dt.int32
AF = mybir.ActivationFunctionType
OP = mybir.AluOpType
AX = mybir.AxisListType

B, T, H, D, K, V, L = 2, 2048, 4, 256, 1024, 32000, 6
VS = V // 4          # vocab shard per core within a b-group
EPS = 1e-5
NT = T // 128        # 16 token chunks
NKC = K // 128       # 8 k chunks
ND = D // 128        # 2 d chunks
TH = T // 2          # 1024

N_LAYERS = int(os.environ.get("KRN_LAYERS", str(L)))
DO_READOUT = os.environ.get("KRN_READOUT", "1") == "1"


def build(nc):
    # ---- DRAM parameters (per core) ----
    tok_d = nc.dram_tensor("tok", [T], I32, kind="ExternalInput")
    emb_d = nc.dram_tensor("emb", [V, D], F32, kind="ExternalInput")
    pos_d = nc.dram_tensor("posb", [T, D], BF16, kind="ExternalInput")
    dx_d = nc.dram_tensor("dxb", [D, K], BF16, kind="ExternalInput")
    dy_d = nc.dram_tensor("dyb", [D, K], BF16, kind="ExternalInput")
    e_d = nc.dram_tensor("eb", [K, D], BF16, kind="ExternalInput")
    ro_d = nc.dram_tensor("rob", [D, VS], BF16, kind="ExternalInput")
    cos_d = nc.dram_tensor("cosb", [8, 128, TH], BF16, kind="ExternalInput")
    sin_d = nc.dram_tensor("sinb", [8, 128, TH], BF16, kind="ExternalInput")
    out_d = nc.dram_tensor("logitsT", [VS, T], BF16, kind="ExternalOutput")

    groups = [[0, 1, 2, 3], [4, 5, 6, 7]]

    with tile.TileContext(nc) as tc:
        with (
            nc.allow_low_precision(reason="bf16 matmul path is intentional"),
            tc.tile_pool(name="persist", bufs=1) as pp,
            tc.tile_pool(name="w4", bufs=4) as w4p,     # [128,1024] bf16 rope
            tc.tile_pool(name="stats", bufs=24) as stp, # [128,8] f32
            tc.tile_pool(name="rop", bufs=3) as rop,    # readout weights
            tc.tile_pool(name="lop", bufs=3) as lop,    # logit staging
            tc.tile_pool(name="pb", bufs=2, space="PSUM") as pbp,   # [128,1024] f32
            tc.tile_pool(name="pa", bufs=2, space="PSUM") as pap,   # [128,256] f32
            tc.tile_pool(name="pt", bufs=2, space="PSUM") as ptp,   # [128,1024] bf16
            tc.tile_pool(name="dram", bufs=1, space="DRAM") as dpool,
        ):
            _ctr = [0]

            def _nm(p):
                _ctr[0] += 1
                return f"{p}{_ctr[0]}"

            # ---- constants ----
            ident_f = pp.tile([128, 128], F32)
            make_identity(nc, ident_f[:])
            ident_bf = pp.tile([128, 128], BF16)
            nc.vector.tensor_copy(ident_bf[:], ident_f[:])
            eps_p = pp.tile([128, 1], F32)
            nc.vector.memset(eps_p[:], EPS)
            nc.gpsimd.load_library(library_config.attn)

            # ---- persistent tensors ----
            v_td = pp.tile([128, NT, D], F32)          # ln(w), token-major (no pos)
            vp_bf = pp.tile([128, NT, D], BF16)        # v + pos, bf16
            vpT = pp.tile([128, ND, T], BF16)          # (v+pos) transposed
            qT = pp.tile([128, NKC, T], BF16)          # q k-major; reused as yt
            qtk = pp.tile([128, NT, K], BF16)          # q token-major
            x_bf = pp.tile([128, NKC, T], BF16)        # relu(v@Dx), k-major
            g0_sb = pp.tile([128, NKC, D], BF16)       # G partial (th0 tokens)
            g_bf = pp.tile([128, NKC, D], BF16)        # G = q^T (v+pos), full
            lnA_dT = pp.tile([128, ND, T], BF16)       # ln(a) d-major
            pos_sb = pp.tile([128, NT, D], BF16)
            dx_sb = pp.tile([128, ND, K], BF16)
            dy_sb = pp.tile([128, ND, K], BF16)
            e_sb = pp.tile([128, NKC, D], BF16)
            # half-batch scratch ([128, 8, 256] = one token half). Multi-purpose;
            # phase ordering keeps uses disjoint (WAR tracked by the framework).
            a0_all = pp.tile([128, 8, D], BF16)        # a half0 / u staging
            a1_all = pp.tile([128, 8, D], BF16)        # a half1 / allreduced u
            uln_all = pp.tile([128, 8, D], F32)        # ln(u) / embed gathers
            lnt_all = pp.tile([128, 8, D], BF16)       # ln(a) chunks / square sink

            nc.sync.dma_start(pos_sb[:], pos_d.ap().rearrange("(j p) d -> p j d", p=128))
            nc.sync.dma_start(dx_sb[:], dx_d.ap().rearrange("(c p) k -> p c k", p=128))
            nc.sync.dma_start(dy_sb[:], dy_d.ap().rearrange("(c p) k -> p c k", p=128))
            nc.sync.dma_start(e_sb[:], e_d.ap().rearrange("(c p) d -> p c d", p=128))

            # ---- internal DRAM (collective staging, bf16) ----
            cc_in = [dpool.tile([8, 128, D], BF16, tag=f"cci{i}", name=f"cci{i}")
                     for i in range(2)]
            cc_out = [dpool.tile([8, 128, D], BF16, tag=f"cco{i}", name=f"cco{i}")
                      for i in range(2)]

            def stats_tail(sum_t, sq_t):
                """Scalar chain from per-chunk sums/sumsqs: (rstd, nmr) [128,8]."""
                negm = stp.tile([128, 8], F32, tag="st", name=_nm("st_"))
                nc.vector.tensor_scalar_mul(negm[:], sum_t[:], -1.0 / D)
                msq = stp.tile([128, 8], F32, tag="st", name=_nm("st_"))
                nc.vector.tensor_mul(msq[:], negm[:], negm[:])
                var = stp.tile([128, 8], F32, tag="st", name=_nm("st_"))
                nc.vector.scalar_tensor_tensor(
                    out=var[:], in0=sq_t[:], scalar=1.0 / D, in1=msq[:],
                    op0=OP.mult, op1=OP.subtract)
                sd = stp.tile([128, 8], F32, tag="st", name=_nm("st_"))
                nc.scalar.activation(sd[:], var[:], AF.Sqrt, bias=eps_p[:])
                rstd = stp.tile([128, 8], F32, tag="st", name=_nm("st_"))
                nc.vector.reciprocal(rstd[:], sd[:])
                nmr = stp.tile([128, 8], F32, tag="st", name=_nm("st_"))
                nc.vector.tensor_mul(nmr[:], negm[:], rstd[:])
                return rstd, nmr

            def sq_dve(src3d_ap, sq_t):
                """sumsq via DVE mul into lnt_all then DVE reduce (short chain)."""
                nc.vector.tensor_mul(lnt_all[:], src3d_ap, src3d_ap)
                nc.vector.tensor_reduce(sq_t[:], lnt_all[:], axis=AX.X, op=OP.add)

            def sq_tile():
                return stp.tile([128, 8], BF16, tag="stb", name=_nm("st_"))

            def apply_half(dst_tile, dst0, src_tile, src0, rstd, nmr):
                for j8 in range(8):
                    nc.scalar.activation(dst_tile[:, dst0 + j8],
                                         src_tile[:, src0 + j8], AF.Identity,
                                         bias=nmr[:, j8:j8 + 1],
                                         scale=rstd[:, j8:j8 + 1])

            def transpose_half(src_tile, sl, dst, c0):
                """Transpose 8 [128, 256] td chunks into dst[:, dc, c0:c0+1024]."""
                tpa = ptp.tile([128, TH], BF16, tag="pt", name=_nm("pt_"))
                tpb = ptp.tile([128, TH], BF16, tag="pt", name=_nm("pt_"))
                for j8 in range(8):
                    nc.tensor.transpose(tpa[:, j8 * 128:(j8 + 1) * 128],
                                        src_tile[:, sl + j8, 0:128], ident_bf[:])
                    nc.tensor.transpose(tpb[:, j8 * 128:(j8 + 1) * 128],
                                        src_tile[:, sl + j8, 128:256], ident_bf[:])
                nc.scalar.copy(dst[:, 0, c0:c0 + TH], tpa[:])
                nc.scalar.copy(dst[:, 1, c0:c0 + TH], tpb[:])

            # ================= embedding gather + LN; vp = ln + pos ==========
            idx = pp.tile([128, NT], I32)
            nc.sync.dma_start(idx[:], tok_d.ap().rearrange("(n p) -> p n", p=128))
            for th in range(2):
                h0 = th * 8
                sl = slice(h0, h0 + 8)
                for j8 in range(8):
                    nc.gpsimd.indirect_dma_start(
                        out=uln_all[:, j8], out_offset=None, in_=emb_d.ap(),
                        in_offset=bass.IndirectOffsetOnAxis(
                            ap=idx[:, h0 + j8:h0 + j8 + 1], axis=0),
                    )
                sum_t = stp.tile([128, 8], F32, tag="st", name=_nm("st_"))
                nc.vector.tensor_reduce(sum_t[:], uln_all[:], axis=AX.X, op=OP.add)
                sq_t = sq_tile()
                sq_dve(uln_all[:], sq_t)
                rstd, nmr = stats_tail(sum_t, sq_t)
                apply_half(v_td, h0, uln_all, 0, rstd, nmr)
                nc.vector.tensor_add(vp_bf[:, sl], v_td[:, sl], pos_sb[:, sl])
                transpose_half(vp_bf, h0, vpT, th * TH)

            def phaseB(th):
                """x[:, th cols] = relu(vp @ Dx); RoPE -> q; build qtk."""
                c0 = th * TH

                def px_one(i):
                    px = pbp.tile([128, TH], F32, tag="pb", name=_nm("pb_"))
                    for dc in range(ND):
                        for ns in range(2):
                            nc.tensor.matmul(
                                px[:, ns * 512:(ns + 1) * 512],
                                dx_sb[:, dc, i * 128:(i + 1) * 128],
                                vpT[:, dc, c0 + ns * 512:c0 + (ns + 1) * 512],
                                start=(dc == 0), stop=(dc == ND - 1))
                    nc.scalar.activation(x_bf[:, i, c0:c0 + TH], px[:], AF.Relu)

                def rope_one(i):
                    cos_t = w4p.tile([128, TH], BF16, tag="w4", name=_nm("w4_"))
                    nc.sync.dma_start(cos_t[:], cos_d.ap()[i * 2 + th])
                    sin_t = w4p.tile([128, TH], BF16, tag="w4", name=_nm("w4_"))
                    nc.sync.dma_start(sin_t[:], sin_d.ap()[i * 2 + th])
                    xi = x_bf[:, i, c0:c0 + TH]
                    xj = x_bf[:, i + 4, c0:c0 + TH]
                    ma = w4p.tile([128, TH], BF16, tag="w4", name=_nm("w4_"))
                    nc.vector.tensor_mul(ma[:], xi, cos_t[:])
                    mb = w4p.tile([128, TH], BF16, tag="w4", name=_nm("w4_"))
                    nc.vector.tensor_mul(mb[:], xj, sin_t[:])
                    nc.vector.tensor_sub(qT[:, i, c0:c0 + TH], ma[:], mb[:])
                    nc.vector.tensor_mul(ma[:], xj, cos_t[:])
                    nc.vector.tensor_mul(mb[:], xi, sin_t[:])
                    nc.vector.tensor_add(qT[:, i + 4, c0:c0 + TH], ma[:], mb[:])

                # pair-interleaved so rope(i) can start while px continues
                px_one(0); px_one(4); rope_one(0)
                px_one(1); px_one(5); rope_one(1)
                px_one(2); px_one(6); rope_one(2)
                px_one(3); px_one(7); rope_one(3)
                for j in range(th * 8, th * 8 + 8):
                    tq = ptp.tile([128, K], BF16, tag="pt", name=_nm("pt_"))
                    for kc in range(NKC):
                        nc.tensor.transpose(tq[:, kc * 128:(kc + 1) * 128],
                                            qT[:, kc, j * 128:(j + 1) * 128],
                                            ident_bf[:])
                    nc.vector.tensor_copy(qtk[:, j], tq[:])

            def phaseG(half):
                """G half-accumulation over token chunks; half 1 finalizes g_bf."""
                for kc in range(NKC):
                    pg = pap.tile([128, D], F32, tag="pa", name=_nm("pa_"))
                    for j in range(half * 8, half * 8 + 8):
                        nc.tensor.matmul(pg[:], qtk[:, j, kc * 128:(kc + 1) * 128],
                                         vp_bf[:, j],
                                         start=(j == half * 8), stop=(j == half * 8 + 7))
                    if half == 0:
                        nc.scalar.copy(g0_sb[:, kc], pg[:])
                    else:
                        nc.vector.tensor_add(g_bf[:, kc], g0_sb[:, kc], pg[:])

            def phaseCa_mm(half, dst_tile, sum_t):
                """a = q G matmuls for a token half; copies carry the sums."""
                h0 = half * 8
                for j8 in range(8):
                    j = h0 + j8
                    paa = pap.tile([128, D], F32, tag="pa", name=_nm("pa_"))
                    for kc in range(NKC):
                        nc.tensor.matmul(paa[:], qT[:, kc, j * 128:(j + 1) * 128],
                                         g_bf[:, kc],
                                         start=(kc == 0), stop=(kc == NKC - 1))
                    nc.scalar.activation(dst_tile[:, j8], paa[:], AF.Copy,
                                         accum_out=sum_t[:, j8:j8 + 1])

            def phaseCa_fin(half, src_tile, sum_t):
                """Batched LN of the a half -> lnA_dT."""
                sq_t = sq_tile()
                sq_dve(src_tile[:], sq_t)
                rstd, nmr = stats_tail(sum_t, sq_t)
                apply_half(lnt_all, 0, src_tile, 0, rstd, nmr)
                transpose_half(lnt_all, 0, lnA_dT, half * TH)

            def phaseDy(th):
                """y = relu(lnA@Dy)*x into yt (aliases q's buffer)."""
                c0 = th * TH
                yt = qT
                for i in range(NKC):
                    py = pbp.tile([128, TH], F32, tag="pb", name=_nm("pb_"))
                    for dc in range(ND):
                        for ns in range(2):
                            nc.tensor.matmul(
                                py[:, ns * 512:(ns + 1) * 512],
                                dy_sb[:, dc, i * 128:(i + 1) * 128],
                                lnA_dT[:, dc, c0 + ns * 512:c0 + (ns + 1) * 512],
                                start=(dc == 0), stop=(dc == ND - 1))
                    nc.vector.scalar_tensor_tensor(
                        out=yt[:, i, c0:c0 + TH], in0=py[:], scalar=0.0,
                        in1=x_bf[:, i, c0:c0 + TH], op0=OP.max, op1=OP.mult)

            def phaseDu(th):
                """u = y@E (token-major); stage bf16 (a0_all) and AllReduce."""
                yt = qT
                for j8 in range(8):
                    j = th * 8 + j8
                    pu = pap.tile([128, D], F32, tag="pa", name=_nm("pa_"))
                    for i in range(NKC):
                        nc.tensor.matmul(pu[:], yt[:, i, j * 128:(j + 1) * 128],
                                         e_sb[:, i],
                                         start=(i == 0), stop=(i == NKC - 1))
                    nc.scalar.copy(a0_all[:, j8], pu[:])
                nc.sync.dma_start(
                    cc_in[th][:].rearrange("j p d -> p j d"), a0_all[:])
                nc.gpsimd.collective_compute(
                    "AllReduce", OP.add, replica_groups=groups,
                    ins=[cc_in[th][:].opt()], outs=[cc_out[th][:].opt()])

            def phaseE_pre(th, ubuf):
                """+pos (collective-independent) and the u DMA; never blocks
                the DVE queue on the collective."""
                sl = slice(th * 8, th * 8 + 8)
                nc.vector.tensor_add(v_td[:, sl], v_td[:, sl], pos_sb[:, sl])
                nc.sync.dma_start(ubuf[:],
                                  cc_out[th][:].rearrange("j p d -> p j d"))

            def phaseE_stats(th, layer, ubuf):
                """w = (v+pos)+ln(u); v = ln(w); vp = v+pos' (no PE work)."""
                last = layer == N_LAYERS - 1
                h0 = th * 8
                sl = slice(h0, h0 + 8)
                nc.vector.tensor_mul(lnt_all[:], ubuf[:], ubuf[:])
                sum_t = stp.tile([128, 8], BF16, tag="stb", name=_nm("st_"))
                nc.vector.tensor_reduce(sum_t[:], ubuf[:], axis=AX.X, op=OP.add)
                sq_t = stp.tile([128, 8], BF16, tag="stb", name=_nm("st_"))
                nc.vector.tensor_reduce(sq_t[:], lnt_all[:], axis=AX.X, op=OP.add)
                rstd, nmr = stats_tail(sum_t, sq_t)
                apply_half(uln_all, 0, ubuf, 0, rstd, nmr)
                nc.vector.tensor_add(v_td[:, sl], v_td[:, sl], uln_all[:])
                nc.vector.tensor_mul(lnt_all[:], v_td[:, sl], v_td[:, sl])
                sum_w = stp.tile([128, 8], BF16, tag="stb", name=_nm("st_"))
                nc.vector.tensor_reduce(sum_w[:], v_td[:, sl], axis=AX.X, op=OP.add)
                sq_w = stp.tile([128, 8], BF16, tag="stb", name=_nm("st_"))
                nc.vector.tensor_reduce(sq_w[:], lnt_all[:], axis=AX.X, op=OP.add)
                rstd_w, nmr_w = stats_tail(sum_w, sq_w)
                apply_half(v_td, h0, v_td, h0, rstd_w, nmr_w)
                if not last:
                    nc.vector.tensor_add(vp_bf[:, sl], v_td[:, sl], pos_sb[:, sl])
                else:
                    nc.scalar.copy(vp_bf[:, sl], v_td[:, sl])

            def phaseE_tp(th):
                transpose_half(vp_bf, th * 8, vpT, th * TH)

            def readout_half(th):
                """logitsT[:, th cols] = (v @ readout)^T for the token half."""
                nvb = (VS + 127) // 128
                for vb in range(nvb):
                    m = min(128, VS - vb * 128)
                    rot = rop.tile([128, ND, 128], BF16, tag="ro", name=_nm("ro_"))
                    for dc in range(ND):
                        nc.sync.dma_start(
                            rot[:, dc, :m],
                            ro_d.ap()[dc * 128:(dc + 1) * 128,
                                      vb * 128:vb * 128 + m])
                    pl = pbp.tile([128, TH], F32, tag="pb", name=_nm("pb_"))
                    for dc in range(ND):
                        for ns in range(2):
                            nc.tensor.matmul(
                                pl[:m, ns * 512:(ns + 1) * 512],
                                rot[:, dc, :m],
                                vpT[:, dc, th * TH + ns * 512:
                                    th * TH + (ns + 1) * 512],
                                start=(dc == 0), stop=(dc == ND - 1))
                    lo = lop.tile([128, TH], BF16, tag="lo", name=_nm("lo_"))
                    if vb % 2 == 0:
                        nc.vector.tensor_copy(lo[:m], pl[:m])
                    else:
                        nc.scalar.copy(lo[:m], pl[:m])
                    nc.sync.dma_start(
                        out_d.ap()[vb * 128:vb * 128 + m, th * TH:(th + 1) * TH],
                        lo[:m])

            # ================================ layers ================================
            phaseB(0)
            phaseB(1)
            phaseG(0)
            phaseG(1)
            for layer in range(N_LAYERS):
                last = layer == N_LAYERS - 1
                with nc.named_scope(f"L{layer}"):
                    sum_a0 = stp.tile([128, 8], F32, tag="st", name=_nm("st_"))
                    sum_a1 = stp.tile([128, 8], F32, tag="st", name=_nm("st_"))
                    phaseCa_mm(0, a0_all, sum_a0)
                    phaseCa_mm(1, a1_all, sum_a1)   # PE covers half0's LN stats
                    phaseCa_fin(0, a0_all, sum_a0)
                    phaseDy(0)
                    phaseCa_fin(1, a1_all, sum_a1)  # stats overlap Dy(0)
                    phaseDu(0)              # cc0 in flight...
                    phaseE_pre(0, a1_all)   # u0 lands in a1_all
                    phaseDy(1)
                    phaseDu(1)              # cc1 in flight...
                    phaseE_stats(0, layer, a1_all)
                    phaseE_pre(1, a0_all)   # u1 lands in a0_all
                    phaseE_tp(0)
                    if not last:
                        phaseB(0)           # B/G(th0) cover cc1's flight
                        phaseG(0)
                        phaseE_stats(1, layer, a0_all)
                        phaseE_tp(1)
                        phaseB(1)
                        phaseG(1)
                    else:
                        if DO_READOUT:
                            readout_half(0)
                        phaseE_stats(1, layer, a0_all)
                        phaseE_tp(1)
                        if DO_READOUT:
                            readout_half(1)

    nc.compile()
    return nc


_NC_CACHE = None


def _get_nc():
    global _NC_CACHE
    if _NC_CACHE is None:
        nc = bacc.Bacc("TRN2", target_bir_lowering=False, debug=False, num_devices=8)
        _NC_CACHE = build(nc)
    return _NC_CACHE


def _rope_tables():
    # match the jax reference: float32 angle computation, then bf16 cast
    import ml_dtypes
    inv_freq = (1.0 / (10000.0 ** (np.arange(0, K, 2, dtype=np.float32)
                                   / np.float32(K)))).astype(np.float32)
    t = np.arange(T, dtype=np.float32)
    freqs = (t[:, None] * inv_freq[None, :]).astype(np.float32)  # [T, K/2]
    cos = np.cos(freqs).astype(np.float32)
    sin = np.sin(freqs).astype(np.float32)
    # [K/2, T] -> [4, 128, 2, 1024] -> [8, 128, 1024] with index i*2+th
    def pack(a):
        aT = np.ascontiguousarray(a.T).reshape(4, 128, 2, TH)
        return np.ascontiguousarray(
            aT.transpose(0, 2, 1, 3).reshape(8, 128, TH)).astype(ml_dtypes.bfloat16)
    return pack(cos), pack(sin)


def kernel(input_, emb, pos, Dx, Dy, E, readout):
    import ml_dtypes
    BF = ml_dtypes.bfloat16
    input_ = np.asarray(input_)
    emb = np.ascontiguousarray(np.asarray(emb, dtype=np.float32))
    pos = np.ascontiguousarray(np.asarray(pos, dtype=np.float32))
    Dx = np.asarray(Dx, dtype=np.float32)
    Dy = np.asarray(Dy, dtype=np.float32)
    E = np.asarray(E, dtype=np.float32)
    readout = np.asarray(readout, dtype=np.float32)

    nc = _get_nc()
    cosb, sinb = _rope_tables()
    ro_bf = readout.astype(BF)

    in_maps = []
    for c in range(8):
        b, h = divmod(c, 4)
        in_maps.append({
            "tok": np.ascontiguousarray(input_[b].astype(np.int32)),
            "emb": emb,
            "posb": np.ascontiguousarray(pos.astype(BF)),
            "dxb": np.ascontiguousarray(Dx[h].astype(BF)),
            "dyb": np.ascontiguousarray(Dy[h].astype(BF)),
            "eb": np.ascontiguousarray(E[h * K:(h + 1) * K].astype(BF)),
            "rob": np.ascontiguousarray(ro_bf[:, h * VS:(h + 1) * VS]),
            "cosb": cosb,
            "sinb": sinb,
        })
    trace = os.environ.get("KRN_TRACE", "0") == "1"
    res = run_bass_kernel_spmd(nc, in_maps, list(range(8)), trace=trace)
    out = np.empty((B, T, V), dtype=np.float32)
    for c in range(8):
        b, h = divmod(c, 4)
        out[b, :, h * VS:(h + 1) * VS] = res.results[c]["logitsT"].astype(np.float32).T
    kernel._last_results = res
    return out


# revision 22
# speedup vs baseline: 1.1492x; 1.0075x over previous
"""Trainium2 Bass kernel for nn_BDH_4406636445711 (dense transformer).

Sharding: 8 cores = data-parallel over B(2) x tensor-parallel over H(4).
Core c handles (b = c//4, h = c%4): its head's Dx/Dy slices, E rows, and a
V/4 shard of the readout. Per layer the y@E partial is AllReduced (bf16)
within each b-group of 4 cores. The host stitches the 8 per-core [VS, T]
logit shards (bf16 on device, cast to fp32 host-side) into [B, T, V].

Key algebraic optimization vs the naive graph: scores = q @ q^T is only
ever used for a = scores @ v, so we compute a = q @ (q^T v) instead --
G = q^T v is [K, D]; ~5x fewer PE cycles than materializing [T, T] scores.

Layouts: v lives token-major ("td": [128 tok part, 16 chunk, 256 d]) so
every LayerNorm is a free-dim reduction. x/q live kT; q is additionally
transposed to tk for the G matmul. All matmul operands are bf16 (full PE
rate); accumulation and LN stats stay fp32. ACT only ever needs the
{relu, copy, sqrt, square, identity} table -- no table reloads.

Scheduling notes:
- Both a-halves' matmuls are issued back-to-back so the first half's
  batched LN statistics overlap the second half's matmuls; the a-sums
  ride for free on the PSUM->SBUF copies via the ACT accumulator.
- v_td holds ln(w) WITHOUT pos; the +pos happens (a) fused into the
  vp_bf cast (one DVE add, bf16 out) and (b) at the top of the next
  E chain where it is latency-hidden. This shortens the E tail that
  gates the next phase-B matmuls.
- The layer is software-pipelined around the two AllReduces: E(th0)'s
  stat chain overlaps D(th1)'s matmuls; the next layer's B/G halves
  cover the second collective. The readout runs as two column-half
  passes so the first pass overlaps the last layer's E(th1) chain.
"""

import os
import sys

sys.path.insert(0, "/opt/trn_rl_repo")

import numpy as np

import concourse.bass as bass
import concourse.tile as tile
from concourse import bacc, mybir
from concourse.bass_utils import run_bass_kernel_spmd
from concourse.masks import make_identity
from concourse import library_config

F32 = mybir.dt.float32
BF16 = mybir.dt.bfloat16
I32 = mybir.dt.int32
AF = mybir.ActivationFunctionType
OP = mybir.AluOpType
AX = mybir.AxisListType

B, T, H, D, K, V, L = 2, 2048, 4, 256, 1024, 32000, 6
VS = V // 4          # vocab shard per core within a b-group
EPS = 1e-5
NT = T // 128        # 16 token chunks
NKC = K // 128       # 8 k chunks
ND = D // 128        # 2 d chunks
TH = T // 2          # 1024

N_LAYERS = int(os.environ.get("KRN_LAYERS", str(L)))
DO_READOUT = os.environ.get("KRN_READOUT", "1") == "1"


def build(nc):
    # ---- DRAM parameters (per core) ----
    tok_d = nc.dram_tensor("tok", [T], I32, kind="ExternalInput")
    emb_d = nc.dram_tensor("emb", [V, D], F32, kind="ExternalInput")
    pos_d = nc.dram_tensor("posb", [T, D], BF16, kind="ExternalInput")
    dx_d = nc.dram_tensor("dxb", [D, K], BF16, kind="ExternalInput")
    dy_d = nc.dram_tensor("dyb", [D, K], BF16, kind="ExternalInput")
    e_d = nc.dram_tensor("eb", [K, D], BF16, kind="ExternalInput")
    ro_d = nc.dram_tensor("rob", [D, VS], BF16, kind="ExternalInput")
    cos_d = nc.dram_tensor("cosb", [8, 128, TH], BF16, kind="ExternalInput")
    sin_d = nc.dram_tensor("sinb", [8, 128, TH], BF16, kind="ExternalInput")
    out_d = nc.dram_tensor("logitsT", [VS, T], BF16, kind="ExternalOutput")

    groups = [[0, 1, 2, 3], [4, 5, 6, 7]]

    with tile.TileContext(nc) as tc:
        with (
            nc.allow_low_precision(reason="bf16 matmul path is intentional"),
            tc.tile_pool(name="persist", bufs=1) as pp,
            tc.tile_pool(name="w4", bufs=4) as w4p,     # [128,1024] bf16 rope
            tc.tile_pool(name="stats", bufs=24) as stp, # [128,8] f32
            tc.tile_pool(name="rop", bufs=3) as rop,    # readout weights
            tc.tile_pool(name="lop", bufs=3) as lop,    # logit staging
            tc.tile_pool(name="pb", bufs=2, space="PSUM") as pbp,   # [128,1024] f32
            tc.tile_pool(name="pa", bufs=2, space="PSUM") as pap,   # [128,256] f32
            tc.tile_pool(name="pt", bufs=2, space="PSUM") as ptp,   # [128,1024] bf16
            tc.tile_pool(name="dram", bufs=1, space="DRAM") as dpool,
        ):
            _ctr = [0]

            def _nm(p):
                _ctr[0] += 1
                return f"{p}{_ctr[0]}"

            # ---- constants ----
            ident_f = pp.tile([128, 128], F32)
            make_identity(nc, ident_f[:])
            ident_bf = pp.tile([128, 128], BF16)
            nc.vector.tensor_copy(ident_bf[:], ident_f[:])
            eps_p = pp.tile([128, 1], F32)
            nc.vector.memset(eps_p[:], EPS)
            nc.gpsimd.load_library(library_config.attn)

            # ---- persistent tensors ----
            v_td = pp.tile([128, NT, D], F32)          # ln(w), token-major (no pos)
            vp_bf = pp.tile([128, NT, D], BF16)        # v + pos, bf16
            vpT = pp.tile([128, ND, T], BF16)          # (v+pos) transposed
            qT = pp.tile([128, NKC, T], BF16)          # q k-major; reused as yt
            qtk = pp.tile([128, NT, K], BF16)          # q token-major
            x_bf = pp.tile([128, NKC, T], BF16)        # relu(v@Dx), k-major
            g0_sb = pp.tile([128, NKC, D], BF16)       # G partial (th0 tokens)
            g_bf = pp.tile([128, NKC, D], BF16)        # G = q^T (v+pos), full
            lnA_dT = pp.tile([128, ND, T], BF16)       # ln(a) d-major
            pos_sb = pp.tile([128, NT, D], BF16)
            dx_sb = pp.tile([128, ND, K], BF16)
            dy_sb = pp.tile([128, ND, K], BF16)
            e_sb = pp.tile([128, NKC, D], BF16)
            # half-batch scratch ([128, 8, 256] = one token half). Multi-purpose;
            # phase ordering keeps uses disjoint (WAR tracked by the framework).
            a0_all = pp.tile([128, 8, D], BF16)        # a half0 / u staging
            a1_all = pp.tile([128, 8, D], BF16)        # a half1 / allreduced u
            uln_all = pp.tile([128, 8, D], F32)        # ln(u) / embed gathers
            lnt_all = pp.tile([128, 8, D], BF16)       # ln(a) chunks / square sink

            nc.sync.dma_start(pos_sb[:], pos_d.ap().rearrange("(j p) d -> p j d", p=128))
            nc.sync.dma_start(dx_sb[:], dx_d.ap().rearrange("(c p) k -> p c k", p=128))
            nc.sync.dma_start(dy_sb[:], dy_d.ap().rearrange("(c p) k -> p c k", p=128))
            nc.sync.dma_start(e_sb[:], e_d.ap().rearrange("(c p) d -> p c d", p=128))

            # ---- internal DRAM (collective staging, bf16) ----
            cc_in = [dpool.tile([8, 128, D], BF16, tag=f"cci{i}", name=f"cci{i}")
                     for i in range(2)]
            cc_out = [dpool.tile([8, 128, D], BF16, tag=f"cco{i}", name=f"cco{i}")
                      for i in range(2)]

            def stats_tail(sum_t, sq_t):
                """Scalar chain from per-chunk sums/sumsqs: (rstd, nmr) [128,8]."""
                negm = stp.tile([128, 8], F32, tag="st", name=_nm("st_"))
                nc.vector.tensor_scalar_mul(negm[:], sum_t[:], -1.0 / D)
                msq = stp.tile([128, 8], F32, tag="st", name=_nm("st_"))
                nc.vector.tensor_mul(msq[:], negm[:], negm[:])
                var = stp.tile([128, 8], F32, tag="st", name=_nm("st_"))
                nc.vector.scalar_tensor_tensor(
                    out=var[:], in0=sq_t[:], scalar=1.0 / D, in1=msq[:],
                    op0=OP.mult, op1=OP.subtract)
                sd = stp.tile([128, 8], F32, tag="st", name=_nm("st_"))
                nc.scalar.activation(sd[:], var[:], AF.Sqrt, bias=eps_p[:])
                rstd = stp.tile([128, 8], F32, tag="st", name=_nm("st_"))
                nc.vector.reciprocal(rstd[:], sd[:])
                nmr = stp.tile([128, 8], F32, tag="st", name=_nm("st_"))
                nc.vector.tensor_mul(nmr[:], negm[:], rstd[:])
                return rstd, nmr

            def sq_dve(src3d_ap, sq_t):
                """sumsq via DVE mul into lnt_all then DVE reduce (short chain)."""
                nc.vector.tensor_mul(lnt_all[:], src3d_ap, src3d_ap)
                nc.vector.tensor_reduce(sq_t[:], lnt_all[:], axis=AX.X, op=OP.add)

            def sq_tile():
                return stp.tile([128, 8], BF16, tag="stb", name=_nm("st_"))

            def apply_half(dst_tile, dst0, src_tile, src0, rstd, nmr):
                for j8 in range(8):
                    nc.scalar.activation(dst_tile[:, dst0 + j8],
                                         src_tile[:, src0 + j8], AF.Identity,
                                         bias=nmr[:, j8:j8 + 1],
                                         scale=rstd[:, j8:j8 + 1])

            def transpose_half(src_tile, sl, dst, c0):
                """Transpose 8 [128, 256] td chunks into dst[:, dc, c0:c0+1024]."""
                tpa = ptp.tile([128, TH], BF16, tag="pt", name=_nm("pt_"))
                tpb = ptp.tile([128, TH], BF16, tag="pt", name=_nm("pt_"))
                for j8 in range(8):
                    nc.tensor.transpose(tpa[:, j8 * 128:(j8 + 1) * 128],
                                        src_tile[:, sl + j8, 0:128], ident_bf[:])
                    nc.tensor.transpose(tpb[:, j8 * 128:(j8 + 1) * 128],
                                        src_tile[:, sl + j8, 128:256], ident_bf[:])
                nc.scalar.copy(dst[:, 0, c0:c0 + TH], tpa[:])
                nc.scalar.copy(dst[:, 1, c0:c0 + TH], tpb[:])

            # ================= embedding gather + LN; vp = ln + pos ==========
            idx = pp.tile([128, NT], I32)
            nc.sync.dma_start(idx[:], tok_d.ap().rearrange("(n p) -> p n", p=128))
            for th in range(2):
                h0 = th * 8
                sl = slice(h0, h0 + 8)
                for j8 in range(8):
                    nc.gpsimd.indirect_dma_start(
                        out=uln_all[:, j8], out_offset=None, in_=emb_d.ap(),
                        in_offset=bass.IndirectOffsetOnAxis(
                            ap=idx[:, h0 + j8:h0 + j8 + 1], axis=0),
                    )
                sum_t = stp.tile([128, 8], F32, tag="st", name=_nm("st_"))
                nc.vector.tensor_reduce(sum_t[:], uln_all[:], axis=AX.X, op=OP.add)
                sq_t = sq_tile()
                sq_dve(uln_all[:], sq_t)
                rstd, nmr = stats_tail(sum_t, sq_t)
                apply_half(v_td, h0, uln_all, 0, rstd, nmr)
                nc.vector.tensor_add(vp_bf[:, sl], v_td[:, sl], pos_sb[:, sl])
                transpose_half(vp_bf, h0, vpT, th * TH)

            def phaseB(th):
                """x[:, th cols] = relu(vp @ Dx); RoPE -> q; build qtk."""
                c0 = th * TH

                def px_one(i):
                    px = pbp.tile([128, TH], F32, tag="pb", name=_nm("pb_"))
                    for dc in range(ND):
                        for ns in range(2):
                            nc.tensor.matmul(
                                px[:, ns * 512:(ns + 1) * 512],
                                dx_sb[:, dc, i * 128:(i + 1) * 128],
                                vpT[:, dc, c0 + ns * 512:c0 + (ns + 1) * 512],
                                start=(dc == 0), stop=(dc == ND - 1))
                    nc.scalar.activation(x_bf[:, i, c0:c0 + TH], px[:], AF.Relu)

                def rope_one(i):
                    cos_t = w4p.tile([128, TH], BF16, tag="w4", name=_nm("w4_"))
                    nc.sync.dma_start(cos_t[:], cos_d.ap()[i * 2 + th])
                    sin_t = w4p.tile([128, TH], BF16, tag="w4", name=_nm("w4_"))
                    nc.sync.dma_start(sin_t[:], sin_d.ap()[i * 2 + th])
                    xi = x_bf[:, i, c0:c0 + TH]
                    xj = x_bf[:, i + 4, c0:c0 + TH]
                    ma = w4p.tile([128, TH], BF16, tag="w4", name=_nm("w4_"))
                    nc.vector.tensor_mul(ma[:], xi, cos_t[:])
                    mb = w4p.tile([128, TH], BF16, tag="w4", name=_nm("w4_"))
                    nc.vector.tensor_mul(mb[:], xj, sin_t[:])
                    nc.vector.tensor_sub(qT[:, i, c0:c0 + TH], ma[:], mb[:])
                    nc.vector.tensor_mul(ma[:], xj, cos_t[:])
                    nc.vector.tensor_mul(mb[:], xi, sin_t[:])
                    nc.vector.tensor_add(qT[:, i + 4, c0:c0 + TH], ma[:], mb[:])

                # pair-interleaved so rope(i) can start while px continues
                px_one(0); px_one(4); rope_one(0)
                px_one(1); px_one(5); rope_one(1)
                px_one(2); px_one(6); rope_one(2)
                px_one(3); px_one(7); rope_one(3)
                for j in range(th * 8, th * 8 + 8):
                    tq = ptp.tile([128, K], BF16, tag="pt", name=_nm("pt_"))
                    for kc in range(NKC):
                        nc.tensor.transpose(tq[:, kc * 128:(kc + 1) * 128],
                                            qT[:, kc, j * 128:(j + 1) * 128],
                                            ident_bf[:])
                    nc.vector.tensor_copy(qtk[:, j], tq[:])

            def phaseG(half):
                """G half-accumulation over token chunks; half 1 finalizes g_bf."""
                for kc in range(NKC):
                    pg = pap.tile([128, D], F32, tag="pa", name=_nm("pa_"))
                    for j in range(half * 8, half * 8 + 8):
                        nc.tensor.matmul(pg[:], qtk[:, j, kc * 128:(kc + 1) * 128],
                                         vp_bf[:, j],
                                         start=(j == half * 8), stop=(j == half * 8 + 7))
                    if half == 0:
                        nc.scalar.copy(g0_sb[:, kc], pg[:])
                    else:
                        nc.vector.tensor_add(g_bf[:, kc], g0_sb[:, kc], pg[:])

            def phaseCa_mm(half, dst_tile, sum_t):
                """a = q G matmuls for a token half; copies carry the sums."""
                h0 = half * 8
                for j8 in range(8):
                    j = h0 + j8
                    paa = pap.tile([128, D], F32, tag="pa", name=_nm("pa_"))
                    for kc in range(NKC):
                        nc.tensor.matmul(paa[:], qT[:, kc, j * 128:(j + 1) * 128],
                                         g_bf[:, kc],
                                         start=(kc == 0), stop=(kc == NKC - 1))
                    nc.scalar.activation(dst_tile[:, j8], paa[:], AF.Copy,
                                         accum_out=sum_t[:, j8:j8 + 1])

            def phaseCa_fin(half, src_tile, sum_t):
                """Batched LN of the a half -> lnA_dT."""
                sq_t = sq_tile()
                sq_dve(src_tile[:], sq_t)
                rstd, nmr = stats_tail(sum_t, sq_t)
                apply_half(lnt_all, 0, src_tile, 0, rstd, nmr)
                transpose_half(lnt_all, 0, lnA_dT, half * TH)

            def phaseDy(th):
                """y = relu(lnA@Dy)*x into yt (aliases q's buffer)."""
                c0 = th * TH
                yt = qT
                for i in range(NKC):
                    py = pbp.tile([128, TH], F32, tag="pb", name=_nm("pb_"))
                    for dc in range(ND):
                        for ns in range(2):
                            nc.tensor.matmul(
                                py[:, ns * 512:(ns + 1) * 512],
                                dy_sb[:, dc, i * 128:(i + 1) * 128],
                                lnA_dT[:, dc, c0 + ns * 512:c0 + (ns + 1) * 512],
                                start=(dc == 0), stop=(dc == ND - 1))
                    nc.vector.scalar_tensor_tensor(
                        out=yt[:, i, c0:c0 + TH], in0=py[:], scalar=0.0,
                        in1=x_bf[:, i, c0:c0 + TH], op0=OP.max, op1=OP.mult)

            def phaseDu(th):
                """u = y@E (token-major); stage bf16 (a0_all) and AllReduce."""
                yt = qT
                for j8 in range(8):
                    j = th * 8 + j8
                    pu = pap.tile([128, D], F32, tag="pa", name=_nm("pa_"))
                    for i in range(NKC):
                        nc.tensor.matmul(pu[:], yt[:, i, j * 128:(j + 1) * 128],
                                         e_sb[:, i],
                                         start=(i == 0), stop=(i == NKC - 1))
                    nc.scalar.copy(a0_all[:, j8], pu[:])
                nc.sync.dma_start(
                    cc_in[th][:].rearrange("j p d -> p j d"), a0_all[:])
                nc.gpsimd.collective_compute(
                    "AllReduce", OP.add, replica_groups=groups,
                    ins=[cc_in[th][:].opt()], outs=[cc_out[th][:].opt()])

            def phaseE_pre(th, ubuf):
                """+pos (collective-independent) and the u DMA; never blocks
                the DVE queue on the collective."""
                sl = slice(th * 8, th * 8 + 8)
                nc.vector.tensor_add(v_td[:, sl], v_td[:, sl], pos_sb[:, sl])
                nc.sync.dma_start(ubuf[:],
                                  cc_out[th][:].rearrange("j p d -> p j d"))

            def phaseE_stats(th, layer, ubuf):
                """w = (v+pos)+ln(u); v = ln(w); vp = v+pos' (no PE work)."""
                last = layer == N_LAYERS - 1
                h0 = th * 8
                sl = slice(h0, h0 + 8)
                nc.vector.tensor_mul(lnt_all[:], ubuf[:], ubuf[:])
                sum_t = stp.tile([128, 8], BF16, tag="stb", name=_nm("st_"))
                nc.vector.tensor_reduce(sum_t[:], ubuf[:], axis=AX.X, op=OP.add)
                sq_t = stp.tile([128, 8], BF16, tag="stb", name=_nm("st_"))
                nc.vector.tensor_reduce(sq_t[:], lnt_all[:], axis=AX.X, op=OP.add)
                rstd, nmr = stats_tail(sum_t, sq_t)
                apply_half(uln_all, 0, ubuf, 0, rstd, nmr)
                nc.vector.tensor_add(v_td[:, sl], v_td[:, sl], uln_all[:])
                nc.vector.tensor_mul(lnt_all[:], v_td[:, sl], v_td[:, sl])
                sum_w = stp.tile([128, 8], BF16, tag="stb", name=_nm("st_"))
                nc.vector.tensor_reduce(sum_w[:], v_td[:, sl], axis=AX.X, op=OP.add)
                sq_w = stp.tile([128, 8], BF16, tag="stb", name=_nm("st_"))
                nc.vector.tensor_reduce(sq_w[:], lnt_all[:], axis=AX.X, op=OP.add)
                rstd_w, nmr_w = stats_tail(sum_w, sq_w)
                apply_half(v_td, h0, v_td, h0, rstd_w, nmr_w)
                if not last:
                    nc.vector.tensor_add(vp_bf[:, sl], v_td[:, sl], pos_sb[:, sl])
                else:
                    nc.scalar.copy(vp_bf[:, sl], v_td[:, sl])

            def phaseE_tp(th):
                transpose_half(vp_bf, th * 8, vpT, th * TH)

            def readout_half(th):
                """logitsT[:, th cols] = (v @ readout)^T for the token half."""
                nvb = (VS + 127) // 128
                for vb in range(nvb):
                    m = min(128, VS - vb * 128)
                    rot = rop.tile([128, ND, 128], BF16, tag="ro", name=_nm("ro_"))
                    for dc in range(ND):
                        nc.sync.dma_start(
                            rot[:, dc, :m],
                            ro_d.ap()[dc * 128:(dc + 1) * 128,
                                      vb * 128:vb * 128 + m])
                    pl = pbp.tile([128, TH], F32, tag="pb", name=_nm("pb_"))
                    for dc in range(ND):
                        for ns in range(2):
                            nc.tensor.matmul(
                                pl[:m, ns * 512:(ns + 1) * 512],
                                rot[:, dc, :m],
                                vpT[:, dc, th * TH + ns * 512:
                                    th * TH + (ns + 1) * 512],
                                start=(dc == 0), stop=(dc == ND - 1))
                    lo = lop.tile([128, TH], BF16, tag="lo", name=_nm("lo_"))
                    if vb % 2 == 0:
                        nc.vector.tensor_copy(lo[:m], pl[:m])
                    else:
                        nc.scalar.copy(lo[:m], pl[:m])
                    nc.sync.dma_start(
                        out_d.ap()[vb * 128:vb * 128 + m, th * TH:(th + 1) * TH],
                        lo[:m])

            # ================================ layers ================================
            phaseB(0)
            phaseB(1)
            phaseG(0)
            phaseG(1)
            for layer in range(N_LAYERS):
                last = layer == N_LAYERS - 1
                with nc.named_scope(f"L{layer}"):
                    sum_a0 = stp.tile([128, 8], F32, tag="st", name=_nm("st_"))
                    sum_a1 = stp.tile([128, 8], F32, tag="st", name=_nm("st_"))
                    phaseCa_mm(0, a0_all, sum_a0)
                    phaseCa_mm(1, a1_all, sum_a1)   # PE covers half0's LN stats
                    phaseCa_fin(0, a0_all, sum_a0)
                    phaseDy(0)
                    phaseCa_fin(1, a1_all, sum_a1)  # stats overlap Dy(0)
                    phaseDu(0)              # cc0 in flight...
                    phaseE_pre(0, a1_all)   # u0 lands in a1_all
                    phaseDy(1)
                    phaseDu(1)              # cc1 in flight...
                    phaseE_stats(0, layer, a1_all)
                    phaseE_pre(1, a0_all)   # u1 lands in a0_all
                    phaseE_tp(0)
                    if not last:
                        phaseB(0)           # B/G(th0) cover cc1's flight
                        phaseG(0)
                        phaseE_stats(1, layer, a0_all)
                        phaseE_tp(1)
                        phaseB(1)
                        phaseG(1)
                    else:
                        if DO_READOUT:
                            readout_half(0)
                        phaseE_stats(1, layer, a0_all)
                        phaseE_tp(1)
                        if DO_READOUT:
                            readout_half(1)

    nc.compile()
    return nc


_NC_CACHE = None


def _get_nc():
    global _NC_CACHE
    if _NC_CACHE is None:
        nc = bacc.Bacc("TRN2", target_bir_lowering=False, debug=False, num_devices=8)
        _NC_CACHE = build(nc)
    return _NC_CACHE


def _rope_tables():
    # match the jax reference: float32 angle computation, then bf16 cast
    import ml_dtypes
    inv_freq = (1.0 / (10000.0 ** (np.arange(0, K, 2, dtype=np.float32)
                                   / np.float32(K)))).astype(np.float32)
    t = np.arange(T, dtype=np.float32)
    freqs = (t[:, None] * inv_freq[None, :]).astype(np.float32)  # [T, K/2]
    cos = np.cos(freqs).astype(np.float32)
    sin = np.sin(freqs).astype(np.float32)
    # [K/2, T] -> [4, 128, 2, 1024] -> [8, 128, 1024] with index i*2+th
    def pack(a):
        aT = np.ascontiguousarray(a.T).reshape(4, 128, 2, TH)
        return np.ascontiguousarray(
            aT.transpose(0, 2, 1, 3).reshape(8, 128, TH)).astype(ml_dtypes.bfloat16)
    return pack(cos), pack(sin)


def kernel(input_, emb, pos, Dx, Dy, E, readout):
    import ml_dtypes
    BF = ml_dtypes.bfloat16
    input_ = np.asarray(input_)
    emb = np.ascontiguousarray(np.asarray(emb, dtype=np.float32))
    pos = np.ascontiguousarray(np.asarray(pos, dtype=np.float32))
    Dx = np.asarray(Dx, dtype=np.float32)
    Dy = np.asarray(Dy, dtype=np.float32)
    E = np.asarray(E, dtype=np.float32)
    readout = np.asarray(readout, dtype=np.float32)

    nc = _get_nc()
    cosb, sinb = _rope_tables()
    ro_bf = readout.astype(BF)

    in_maps = []
    for c in range(8):
        b, h = divmod(c, 4)
        in_maps.append({
            "tok": np.ascontiguousarray(input_[b].astype(np.int32)),
            "emb": emb,
            "posb": np.ascontiguousarray(pos.astype(BF)),
            "dxb": np.ascontiguousarray(Dx[h].astype(BF)),
            "dyb": np.ascontiguousarray(Dy[h].astype(BF)),
            "eb": np.ascontiguousarray(E[h * K:(h + 1) * K].astype(BF)),
            "rob": np.ascontiguousarray(ro_bf[:, h * VS:(h + 1) * VS]),
            "cosb": cosb,
            "sinb": sinb,
        })
    trace = os.environ.get("KRN_TRACE", "0") == "1"
    res = run_bass_kernel_spmd(nc, in_maps, list(range(8)), trace=trace)
    out = np.empty((B, T, V), dtype=np.float32)
    for c in range(8):
        b, h = divmod(c, 4)
        out[b, :, h * VS:(h + 1) * VS] = res.results[c]["logitsT"].astype(np.float32).T
    kernel._last_results = res
    return out
